# revision 1
# baseline (speedup 1.0000x reference)
# Causal self-attention (B=4, T=2048, C=1024, 16 heads) on 8 NeuronCores.
# Sharding: core = (batch b = core//2) x (head-group hg = core%2, 8 heads each).
# Each core computes its 8 heads' attention for its batch plus the row-slice of
# the output projection; the host sums the two partial projections per batch.
#
# Self-contained: hardcodes shapes; builds + compiles the Bass program once.

import contextlib

import numpy as np
import ml_dtypes

import concourse.bass as bass
import concourse.tile as tile
from concourse import bacc, mybir
from concourse.bass import AP
from concourse.bass_utils import run_bass_kernel_spmd

F32 = mybir.dt.float32
BF16 = mybir.dt.bfloat16
EXP = mybir.ActivationFunctionType.Exp
IDENT = mybir.ActivationFunctionType.Identity

B, T, C = 4, 2048, 1024
NH, HS = 16, 64
NHPC = 8          # heads per core
D = NHPC * HS     # 512: per-core qkv width
NCORES = 8
TT = T // 128     # 16 token tiles
TC = T // 512     # 4 token chunks
CT = C // 128     # 8 contraction tiles
DT = D // 128     # 4 d-tiles of qT/kT (= head pairs)
VW = 68           # per-head stride in v tile: [v(64) | ones | pad3]

_cache = {}


def _bcast_row(ap, nrep):
    """AP reading a [1, N] slice nrep times (free-dim step-0 broadcast)."""
    return AP(ap.tensor, ap.offset, [ap.ap[0], [0, nrep]] + ap.ap[1:])


def _build():
    nc = bacc.Bacc("TRN2", target_bir_lowering=False, debug=False,
                   num_devices=NCORES)

    xT = nc.dram_tensor("xT", [C, T], BF16, kind="ExternalInput")
    wq = nc.dram_tensor("wq", [C, D], BF16, kind="ExternalInput")
    wk = nc.dram_tensor("wk", [C, D], BF16, kind="ExternalInput")
    wv = nc.dram_tensor("wv", [C, D], BF16, kind="ExternalInput")
    wp = nc.dram_tensor("wp", [D, C], BF16, kind="ExternalInput")
    bq = nc.dram_tensor("bq", [128, DT], F32, kind="ExternalInput")
    bk = nc.dram_tensor("bk", [128, DT], F32, kind="ExternalInput")
    bvbc = nc.dram_tensor("bvbc", [128, D], F32, kind="ExternalInput")
    ypT = nc.dram_tensor("ypT", [C, T], F32, kind="ExternalOutput")

    with tile.TileContext(nc) as tc, contextlib.ExitStack() as ctx:
        cpool = ctx.enter_context(tc.tile_pool(name="consts", bufs=1))
        xpool = ctx.enter_context(tc.tile_pool(name="x", bufs=1))
        wpool = ctx.enter_context(tc.tile_pool(name="w", bufs=1))
        qkpool = ctx.enter_context(tc.tile_pool(name="qk", bufs=1))
        vpool = ctx.enter_context(tc.tile_pool(name="v", bufs=1))
        opool = ctx.enter_context(tc.tile_pool(name="oT", bufs=1))
        ptpool = ctx.enter_context(tc.tile_pool(name="pt", bufs=8))
        ypool = ctx.enter_context(tc.tile_pool(name="y", bufs=4))
        avpool = ctx.enter_context(tc.tile_pool(name="av", bufs=3))
        rspool = ctx.enter_context(tc.tile_pool(name="rs", bufs=6))
        rvpool = ctx.enter_context(tc.tile_pool(name="rv", bufs=4))
        qkv_ps = ctx.enter_context(
            tc.tile_pool(name="qkv_ps", bufs=2, space="PSUM"))
        s_ps = ctx.enter_context(
            tc.tile_pool(name="s_ps", bufs=2, space="PSUM"))
        o_ps = ctx.enter_context(
            tc.tile_pool(name="o_ps", bufs=2, space="PSUM"))

        # ---- input DMAs (ordered by first use; spread over sync/scalar) ----
        wvt = [wpool.tile([128, D], BF16, tag=f"wv{i}", name=f"wv{i}")
               for i in range(CT)]
        for i in range(CT):
            (nc.sync if i % 2 == 0 else nc.scalar).dma_start(
                wvt[i][:], wv.ap()[i * 128:(i + 1) * 128, :])
        xt = [xpool.tile([128, T], BF16, tag=f"xT{i}", name=f"xT{i}")
              for i in range(CT)]
        wqt = [wpool.tile([128, D], BF16, tag=f"wq{i}", name=f"wq{i}")
               for i in range(CT)]
        wkt = [wpool.tile([128, D], BF16, tag=f"wk{i}", name=f"wk{i}")
               for i in range(CT)]

        def dma_x_chunk(cch):
            for i in range(CT):
                (nc.sync if i % 2 == 0 else nc.scalar).dma_start(
                    xt[i][:, cch * 512:(cch + 1) * 512],
                    xT.ap()[i * 128:(i + 1) * 128, cch * 512:(cch + 1) * 512])
        dma_x_chunk(0)
        for i in range(CT):
            nc.sync.dma_start(wqt[i][:], wq.ap()[i * 128:(i + 1) * 128, :])
            nc.scalar.dma_start(wkt[i][:], wk.ap()[i * 128:(i + 1) * 128, :])
        for cch in range(1, TC):
            dma_x_chunk(cch)
        wpt = [wpool.tile([128, C], BF16, tag=f"wp{i}", name=f"wp{i}")
               for i in range(DT)]
        for i in range(DT):
            nc.sync.dma_start(wpt[i][:], wp.ap()[i * 128:(i + 1) * 128, :])
        bqt = cpool.tile([128, DT], F32, tag="bq")
        bkt = cpool.tile([128, DT], F32, tag="bk")
        bvt = cpool.tile([128, D], F32, tag="bv")
        nc.sync.dma_start(bqt[:], bq.ap())
        nc.scalar.dma_start(bkt[:], bk.ap())
        nc.sync.dma_start(bvt[:], bvbc.ap())

        # ---- one-time consts ----
        ones8 = cpool.tile([128, NHPC], F32, tag="ones8")
        nc.vector.memset(ones8[:], 1.0)
        ones_t = cpool.tile([128, 512], F32, tag="ones_t")
        nc.vector.memset(ones_t[:], 1.0)
        masks = []
        for t in range(4):
            mf = cpool.tile([128, 512], F32, tag=f"mf{t}", name=f"mf{t}")
            nc.gpsimd.affine_select(
                mf[:], ones_t[:], pattern=[[1, 512]],
                compare_op=mybir.AluOpType.is_ge, fill=0.0,
                base=-(128 * t), channel_multiplier=-1)
            mb = cpool.tile([128, 512], BF16, tag=f"mb{t}", name=f"mb{t}")
            nc.vector.tensor_copy(mb[:], mf[:])
            masks.append(mb)

        # ---- v = x @ Wv + bv, token-major, [v|ones] per head ----
        # warm-up matmuls: keep PE busy (and HAM warm) during input DMA
        warm_ps = qkv_ps.tile([128, 512], F32, tag="qkv", name="warmps")
        for w in range(24):
            nc.tensor.matmul(
                warm_ps[:], wvt[0][:, 0:128], wvt[0][:, 0:512],
                start=True, stop=True, skip_group_check=True)

        vt = [vpool.tile([128, NHPC * VW], BF16, tag=f"v{j}", name=f"v{j}")
              for j in range(TT)]

        def emit_v(j):
            ps = qkv_ps.tile([128, D], F32, tag="qkv", name="qkvps")
            for ct in range(CT):
                nc.tensor.matmul(
                    ps[:], xt[ct][:, j * 128:(j + 1) * 128], wvt[ct][:],
                    start=(ct == 0), stop=(ct == CT - 1))
            vre = vt[j][:].rearrange("p (h x) -> p h x", h=NHPC)
            nc.vector.tensor_copy(
                vre[:, :, 0:1], ones8[:].rearrange("p (h x) -> p h x", x=1))
            nc.vector.tensor_add(
                vre[:, :, 1:65],
                ps[:].rearrange("p (h x) -> p h x", h=NHPC),
                bvt[:].rearrange("p (h x) -> p h x", h=NHPC))

        # ---- qT/kT/oT tiles ----
        qT = [qkpool.tile([128, T], BF16, tag=f"q{d}", name=f"q{d}")
              for d in range(DT)]
        kT = [qkpool.tile([128, T], BF16, tag=f"k{d}", name=f"k{d}")
              for d in range(DT)]
        oT = [opool.tile([128, T], BF16, tag=f"oT{d}", name=f"oT{d}")
              for d in range(DT)]

        def emit_qk_group(hp, idx):
            c, which = idx // 2, idx % 2
            wt_, bt_, out = ((wqt, bqt, qT), (wkt, bkt, kT))[which]
            ps = qkv_ps.tile([128, 512], F32, tag="qkv", name="qkvps")
            for ct in range(CT):
                nc.tensor.matmul(
                    ps[:], wt_[ct][:, hp * 128:(hp + 1) * 128],
                    xt[ct][:, c * 512:(c + 1) * 512],
                    start=(ct == 0), stop=(ct == CT - 1))
            nc.scalar.activation(
                out[hp][:, c * 512:(c + 1) * 512], ps[:], IDENT,
                bias=bt_[:, hp:hp + 1])

        def emit_proj(c):
            for o in range(CT):
                ps = qkv_ps.tile([128, 512], F32, tag="qkv", name="qkvps")
                for hp in range(DT):
                    nc.tensor.matmul(
                        ps[:], wpt[hp][:, o * 128:(o + 1) * 128],
                        oT[hp][:, c * 512:(c + 1) * 512],
                        start=(hp == 0), stop=(hp == DT - 1))
                ys = ypool.tile([128, 512], F32, tag="y", name="ys")
                nc.vector.tensor_copy(ys[:], ps[:])
                nc.sync.dma_start(
                    ypT.ap()[o * 128:(o + 1) * 128, c * 512:(c + 1) * 512],
                    ys[:])

        # deferred normalization: scale oT chunk piece by 1/rowsum
        def emit_norm(hp, c, rv0, rv1):
            cs = slice(c * 512, (c + 1) * 512)
            nc.vector.reciprocal_approx_fast(rv0[0:64, :], rv0[0:64, :])
            nc.vector.tensor_mul(
                oT[hp][0:64, cs], oT[hp][0:64, cs], rv0[0:64, :])
            nc.vector.reciprocal_approx_fast(rv1[:, :], rv1[:, :])
            nc.vector.tensor_mul(
                oT[hp][64:128, cs], oT[hp][64:128, cs], rv1[64:128, :])
            if hp == DT - 1:
                emit_proj(c)

        pending = []

        def flush_pending(keep=0):
            while len(pending) > keep:
                emit_norm(*pending.pop(0))

        # ---- attention per head pair; qk of next pair interleaved ----
        for c in range(TC):
            for j in range(4 * c, 4 * c + 4):
                emit_v(j)
            emit_qk_group(0, 2 * c)
            emit_qk_group(0, 2 * c + 1)
        for hp in range(DT):
            for c in range(TC):
                if hp + 1 < DT:
                    emit_qk_group(hp + 1, 2 * c)
                    emit_qk_group(hp + 1, 2 * c + 1)
                njt = 4 * c + 4
                op0 = o_ps.tile([128, 512], F32, tag="o", name="ops")
                op1 = o_ps.tile([128, 512], F32, tag="o", name="ops")
                for jt in range(njt):
                    sp = s_ps.tile([128, 1024], F32, tag="s", name="sps")
                    for half in range(2):
                        nc.tensor.matmul(
                            sp[:, half * 512:(half + 1) * 512],
                            kT[hp][half * 64:(half + 1) * 64,
                                   jt * 128:(jt + 1) * 128],
                            qT[hp][half * 64:(half + 1) * 64,
                                   c * 512:(c + 1) * 512],
                            start=True, stop=True)
                    pt = ptpool.tile([128, 1024], BF16, tag="pt", name="pt")
                    nc.scalar.activation(pt[:], sp[:], EXP, scale=0.125)
                    if jt >= 4 * c:
                        t = jt - 4 * c
                        nc.vector.tensor_mul(
                            pt[:, 0:512], pt[:, 0:512], masks[t][:])
                        pv = pt[:, 512:1024]
                        nc.gpsimd.affine_select(
                            pv, pv, pattern=[[1, 512]],
                            compare_op=mybir.AluOpType.is_ge, fill=0.0,
                            base=-(128 * t), channel_multiplier=-1)
                    for half, op in ((0, op0), (1, op1)):
                        h = 2 * hp + half
                        nc.tensor.matmul(
                            op[0:65, :], vt[jt][:, h * VW:h * VW + 65],
                            pt[:, half * 512:(half + 1) * 512],
                            start=(jt == 0), stop=(jt == njt - 1))

                # stage unnormalized O + rowsums out of PSUM (fast release)
                cs = slice(c * 512, (c + 1) * 512)
                rs0 = rspool.tile([128, 512], F32, tag="rs", name="rs")
                rs1 = rspool.tile([128, 512], F32, tag="rs", name="rs")
                av0 = avpool.tile([128, 512], BF16, tag="av", name="av")
                av1 = avpool.tile([128, 512], BF16, tag="av", name="av")
                nc.vector.tensor_copy(av0[0:64, :], op0[0:64, :])
                nc.vector.tensor_copy(av0[64:65, :], op0[64:65, :])
                nc.vector.tensor_copy(rs0[0:1, :], op0[0:1, :])
                nc.vector.tensor_copy(av1[0:64, :], op1[0:64, :])
                nc.vector.tensor_copy(av1[64:65, :], op1[64:65, :])
                nc.vector.tensor_copy(rs1[0:1, :], op1[0:1, :])
                nc.sync.dma_start(oT[hp][0:64, cs], av0[1:65, :])
                nc.sync.dma_start(oT[hp][64:128, cs], av1[1:65, :])
                # rowsum broadcast via gpsimd (fast, no DMA round-trip)
                rv0 = rvpool.tile([128, 512], F32, tag="rv", name="rv")
                rv1 = rvpool.tile([128, 512], F32, tag="rv", name="rv")
                nc.gpsimd.partition_broadcast(rv0[:, :], rs0[0:1, :])
                nc.gpsimd.partition_broadcast(rv1[:, :], rs1[0:1, :])

                # run an EARLIER chunk's reciprocal+scale (deps long done)
                pending.append((hp, c, rv0, rv1))
                flush_pending(keep=2)
        flush_pending()

    nc.compile()
    return nc


def _shard_inputs(x, Wk, bk, Wq, bq, Wv, bv, Wp, bp):
    bf = ml_dtypes.bfloat16
    in_maps = []
    for core in range(NCORES):
        b, hg = core // 2, core % 2
        sl = slice(hg * D, (hg + 1) * D)
        in_maps.append({
            "xT": np.ascontiguousarray(x[b].T).astype(bf),
            "wq": np.ascontiguousarray(Wq[:, sl]).astype(bf),
            "wk": np.ascontiguousarray(Wk[:, sl]).astype(bf),
            "wv": np.ascontiguousarray(Wv[:, sl]).astype(bf),
            "wp": np.ascontiguousarray(Wp[sl, :]).astype(bf),
            "bq": np.ascontiguousarray(
                bq[sl].reshape(DT, 128).T).astype(np.float32),
            "bk": np.ascontiguousarray(
                bk[sl].reshape(DT, 128).T).astype(np.float32),
            "bvbc": np.ascontiguousarray(
                np.broadcast_to(bv[sl], (128, D))).astype(np.float32),
        })
    return in_maps


def kernel(x, Wk, bk, Wq, bq, Wv, bv, Wp, bp, _trace=False, _trace_kwargs=None):
    x, Wk, bk, Wq, bq, Wv, bv, Wp, bp = [
        np.asarray(a) for a in (x, Wk, bk, Wq, bq, Wv, bv, Wp, bp)]
    if "nc" not in _cache:
        _cache["nc"] = _build()
    nc = _cache["nc"]
    in_maps = _shard_inputs(x, Wk, bk, Wq, bq, Wv, bv, Wp, bp)
    kw = dict(_trace_kwargs or {})
    res = run_bass_kernel_spmd(nc, in_maps, core_ids=list(range(NCORES)),
                               trace=_trace, **kw)
    out = np.empty((B, T, C), np.float32)
    for b in range(B):
        yp = res.results[2 * b]["ypT"] + res.results[2 * b + 1]["ypT"]
        out[b] = yp.T + bp[None, :]
    if _trace:
        _cache["last_results"] = res
    return out



# revision 11
# speedup vs baseline: 1.0734x; 1.0734x over previous
# Causal self-attention (B=4, T=2048, C=1024, 16 heads) on 8 NeuronCores.
# Sharding: core = (batch b = core//2) x (head-group hg = core%2, 8 heads each).
# Each core computes its 8 heads' attention for its batch plus the row-slice of
# the output projection; the host sums the two partial projections per batch.
#
# v2: chunk-major schedule. Per 512-token query chunk c: v/qk projections for
# the NEXT chunk and the proj of the PREVIOUS chunk are interleaved as PE
# filler into the ACT-bound attention inner loop; S matmuls run one jt ahead
# of AV; normalization divides straight out of PSUM (rowsum broadcast via
# gpsimd) so PSUM banks recycle fast.
#
# Self-contained: hardcodes shapes; builds + compiles the Bass program once.

import contextlib

import numpy as np
import ml_dtypes

import concourse.bass as bass
import concourse.tile as tile
from concourse import bacc, mybir
from concourse.alu_op_type import AluOpType
from concourse.bass import AP
from concourse.bass_utils import run_bass_kernel_spmd

F32 = mybir.dt.float32
BF16 = mybir.dt.bfloat16
EXP = mybir.ActivationFunctionType.Exp
IDENT = mybir.ActivationFunctionType.Identity

B, T, C = 4, 2048, 1024
NH, HS = 16, 64
NHPC = 8          # heads per core
D = NHPC * HS     # 512: per-core qkv width
NCORES = 8
TT = T // 128     # 16 token tiles
TC = T // 512     # 4 token chunks
CT = C // 128     # 8 contraction tiles
DT = D // 128     # 4 d-tiles of qT/kT (= head pairs)
VW = 68           # per-head stride in v tile: [v(64) | ones | pad3]

_cache = {}


def _build():
    nc = bacc.Bacc("TRN2", target_bir_lowering=False, debug=False,
                   num_devices=NCORES)

    xT = nc.dram_tensor("xT", [C, T], BF16, kind="ExternalInput")
    wq = nc.dram_tensor("wq", [C, D], BF16, kind="ExternalInput")
    wk = nc.dram_tensor("wk", [C, D], BF16, kind="ExternalInput")
    wv = nc.dram_tensor("wv", [C, D], BF16, kind="ExternalInput")
    wp = nc.dram_tensor("wp", [D, C], BF16, kind="ExternalInput")
    bq = nc.dram_tensor("bq", [128, DT], F32, kind="ExternalInput")
    bk = nc.dram_tensor("bk", [128, DT], F32, kind="ExternalInput")
    bvbc = nc.dram_tensor("bvbc", [128, D], F32, kind="ExternalInput")
    ypT = nc.dram_tensor("ypT", [C, T], F32, kind="ExternalOutput")

    with tile.TileContext(nc) as tc, contextlib.ExitStack() as ctx:
        cpool = ctx.enter_context(tc.tile_pool(name="consts", bufs=1))
        xpool = ctx.enter_context(tc.tile_pool(name="x", bufs=1))
        wpool = ctx.enter_context(tc.tile_pool(name="w", bufs=1))
        qkpool = ctx.enter_context(tc.tile_pool(name="qk", bufs=1))
        vpool = ctx.enter_context(tc.tile_pool(name="v", bufs=1))
        opool = ctx.enter_context(tc.tile_pool(name="oT", bufs=1))
        ptpool = ctx.enter_context(tc.tile_pool(name="pt", bufs=8))
        ypool = ctx.enter_context(tc.tile_pool(name="y", bufs=4))
        rvpool = ctx.enter_context(tc.tile_pool(name="rv", bufs=4))
        stpool = ctx.enter_context(tc.tile_pool(name="st", bufs=4))
        recpool = ctx.enter_context(tc.tile_pool(name="rec", bufs=2))
        avpool = ctx.enter_context(tc.tile_pool(name="avb", bufs=4))
        qkv_ps = ctx.enter_context(
            tc.tile_pool(name="qkv_ps", bufs=2, space="PSUM"))
        s_ps = ctx.enter_context(
            tc.tile_pool(name="s_ps", bufs=2, space="PSUM"))
        o_ps = ctx.enter_context(
            tc.tile_pool(name="o_ps", bufs=2, space="PSUM"))

        # ---- input DMAs (ordered by first use; spread over sync/scalar) ----
        wvt = [wpool.tile([128, D], BF16, tag=f"wv{i}", name=f"wv{i}")
               for i in range(CT)]
        for i in range(CT):
            (nc.sync if i % 2 == 0 else nc.scalar).dma_start(
                wvt[i][:], wv.ap()[i * 128:(i + 1) * 128, :])
        xt = [xpool.tile([128, T], BF16, tag=f"xT{i}", name=f"xT{i}")
              for i in range(CT)]
        wqt = [wpool.tile([128, D], BF16, tag=f"wq{i}", name=f"wq{i}")
               for i in range(CT)]
        wkt = [wpool.tile([128, D], BF16, tag=f"wk{i}", name=f"wk{i}")
               for i in range(CT)]

        def dma_x_chunk(cch):
            for i in range(CT):
                (nc.sync if i % 2 == 0 else nc.scalar).dma_start(
                    xt[i][:, cch * 512:(cch + 1) * 512],
                    xT.ap()[i * 128:(i + 1) * 128, cch * 512:(cch + 1) * 512])
        dma_x_chunk(0)
        for i in range(CT):
            nc.sync.dma_start(wqt[i][:], wq.ap()[i * 128:(i + 1) * 128, :])
            nc.scalar.dma_start(wkt[i][:], wk.ap()[i * 128:(i + 1) * 128, :])
        bqt = cpool.tile([128, DT], F32, tag="bq")
        bkt = cpool.tile([128, DT], F32, tag="bk")
        bvt = cpool.tile([128, D], F32, tag="bv")
        nc.sync.dma_start(bqt[:], bq.ap())
        nc.scalar.dma_start(bkt[:], bk.ap())
        nc.sync.dma_start(bvt[:], bvbc.ap())
        for cch in range(1, TC):
            dma_x_chunk(cch)
        wpt = [wpool.tile([128, C], BF16, tag=f"wp{i}", name=f"wp{i}")
               for i in range(DT)]
        for i in range(DT):
            (nc.sync if i % 2 == 0 else nc.scalar).dma_start(
                wpt[i][:], wp.ap()[i * 128:(i + 1) * 128, :])

        # ---- one-time consts ----
        ones8 = cpool.tile([128, NHPC], F32, tag="ones8")
        nc.vector.memset(ones8[:], 1.0)
        masks = {}    # odd t: [128,512] bf16, valid = col >= 128t + p
        masks2 = {}   # even t: [128,1024] bf16, mask duplicated for both heads
        mf = cpool.tile([128, 1024], F32, tag="mf", name="mf")
        for t in range(4):
            nc.vector.memset(mf[:], 1.0)
            nc.gpsimd.affine_select(
                mf[:, 0:512], mf[:, 0:512], pattern=[[1, 512]],
                compare_op=mybir.AluOpType.is_ge, fill=0.0,
                base=-(128 * t), channel_multiplier=-1)
            if t % 2 == 0:
                nc.gpsimd.affine_select(
                    mf[:, 512:1024], mf[:, 512:1024], pattern=[[1, 512]],
                    compare_op=mybir.AluOpType.is_ge, fill=0.0,
                    base=-(128 * t), channel_multiplier=-1)
                mb2 = cpool.tile([128, 1024], BF16, tag=f"mb2{t}",
                                 name=f"mb2{t}")
                nc.vector.tensor_copy(mb2[:], mf[:])
                masks2[t] = mb2
            else:
                mb = cpool.tile([128, 512], BF16, tag=f"mb{t}", name=f"mb{t}")
                nc.vector.tensor_copy(mb[:], mf[:, 0:512])
                masks[t] = mb

        # ---- warm-up matmuls: keep PE busy (and HAM warm) during input DMA
        warm_ps = qkv_ps.tile([128, 512], F32, tag="qkv", name="warmps")
        for w in range(24):
            nc.tensor.matmul(
                warm_ps[:], wvt[0][:, 0:128], wvt[0][:, 0:512],
                start=True, stop=True, skip_group_check=True)

        # ---- persistent tiles ----
        vt = [vpool.tile([128, NHPC * VW], BF16, tag=f"v{j}", name=f"v{j}")
              for j in range(TT)]
        qT = [qkpool.tile([128, T], BF16, tag=f"q{d}", name=f"q{d}")
              for d in range(DT)]
        kT = [qkpool.tile([128, T], BF16, tag=f"k{d}", name=f"k{d}")
              for d in range(DT)]
        oT = [opool.tile([128, T], BF16, tag=f"oT{d}", name=f"oT{d}")
              for d in range(DT)]

        # ---- filler generators: yield once per emitted matmul ----
        def gen_v(j):
            ps = qkv_ps.tile([128, D], F32, tag="qkv", name="qkvps")
            for ct in range(CT):
                nc.tensor.matmul(
                    ps[:], xt[ct][:, j * 128:(j + 1) * 128], wvt[ct][:],
                    start=(ct == 0), stop=(ct == CT - 1))
                yield
            vre = vt[j][:].rearrange("p (h x) -> p h x", h=NHPC)
            nc.vector.tensor_copy(
                vre[:, :, 0:1], ones8[:].rearrange("p (h x) -> p h x", x=1))
            nc.vector.tensor_add(
                vre[:, :, 1:65],
                ps[:].rearrange("p (h x) -> p h x", h=NHPC),
                bvt[:].rearrange("p (h x) -> p h x", h=NHPC))

        def gen_qk(hp, c, which):
            wt_, bt_, out = ((wqt, bqt, qT), (wkt, bkt, kT))[which]
            ps = qkv_ps.tile([128, 512], F32, tag="qkv", name="qkvps")
            for ct in range(CT):
                nc.tensor.matmul(
                    ps[:], wt_[ct][:, hp * 128:(hp + 1) * 128],
                    xt[ct][:, c * 512:(c + 1) * 512],
                    start=(ct == 0), stop=(ct == CT - 1))
                yield
            nc.scalar.activation(
                out[hp][:, c * 512:(c + 1) * 512], ps[:], IDENT,
                bias=bt_[:, hp:hp + 1])

        def gen_proj(c):
            for o in range(CT):
                ps = qkv_ps.tile([128, 512], F32, tag="qkv", name="qkvps")
                for hp in range(DT):
                    nc.tensor.matmul(
                        ps[:], wpt[hp][:, o * 128:(o + 1) * 128],
                        oT[hp][:, c * 512:(c + 1) * 512],
                        start=(hp == 0), stop=(hp == DT - 1))
                    yield
                ys = ypool.tile([128, 512], F32, tag="y", name="ys")
                nc.vector.tensor_copy(ys[:], ps[:])
                nc.sync.dma_start(
                    ypT.ap()[o * 128:(o + 1) * 128, c * 512:(c + 1) * 512],
                    ys[:])

        # two filler queues: F (projections for next chunk; must finish
        # before that chunk's attention) and P (output proj; can linger)
        fillF = []
        fillP = []

        def pump(n):
            for _ in range(n):
                q = fillF if fillF else fillP
                if not q:
                    return
                try:
                    next(q[0])
                except StopIteration:
                    q.pop(0)

        def drain_F():
            while fillF:
                try:
                    next(fillF[0])
                except StopIteration:
                    fillF.pop(0)

        def drain_all():
            drain_F()
            while fillP:
                try:
                    next(fillP[0])
                except StopIteration:
                    fillP.pop(0)

        # ---- attention block for head pair hp, query chunk c ----
        def attention(hp, c):
            njt = 4 * c + 4
            cs = slice(c * 512, (c + 1) * 512)
            op0 = o_ps.tile([128, 512], F32, tag="o", name="ops")
            op1 = o_ps.tile([128, 512], F32, tag="o", name="ops")
            sps = {}

            def emit_S(jt):
                sp = s_ps.tile([128, 1024], F32, tag="s", name="sps")
                for half in range(2):
                    nc.tensor.matmul(
                        sp[:, half * 512:(half + 1) * 512],
                        kT[hp][half * 64:(half + 1) * 64,
                               jt * 128:(jt + 1) * 128],
                        qT[hp][half * 64:(half + 1) * 64, cs],
                        start=True, stop=True)
                sps[jt] = sp

            emit_S(0)
            pcr = 0.0
            for jt in range(njt):
                if jt + 1 < njt:
                    emit_S(jt + 1)
                sp = sps.pop(jt)
                pt = ptpool.tile([128, 1024], BF16, tag="pt", name="pt")
                nc.scalar.activation(pt[:], sp[:], EXP, scale=0.125)
                if jt >= 4 * c:
                    t = jt - 4 * c
                    if t % 2 == 0:
                        nc.vector.tensor_mul(pt[:], pt[:], masks2[t][:])
                    else:
                        nc.vector.tensor_mul(
                            pt[:, 0:512], pt[:, 0:512], masks[t][:])
                        nc.gpsimd.affine_select(
                            pt[:, 512:1024], pt[:, 512:1024],
                            pattern=[[1, 512]],
                            compare_op=mybir.AluOpType.is_ge, fill=0.0,
                            base=-(128 * t), channel_multiplier=-1)
                for half, op in ((0, op0), (1, op1)):
                    h = 2 * hp + half
                    nc.tensor.matmul(
                        op[0:65, :], vt[jt][:, h * VW:h * VW + 65],
                        pt[:, half * 512:(half + 1) * 512],
                        start=(jt == 0), stop=(jt == njt - 1))
                pcr += 0.8
                if pcr >= 1.0:
                    k = int(pcr)
                    pump(k)
                    pcr -= k

            # normalization: stage unnormalized rows out of PSUM (releases
            # the bank), recip the rowsum straight from PSUM partition 0,
            # DMA-shift 1:65 -> oT, then scale oT in place (aligned).
            for half, op, dst in ((0, op0, oT[hp][0:64, cs]),
                                  (1, op1, oT[hp][64:128, cs])):
                av = avpool.tile([128, 512], BF16, tag="avb", name="avb")
                nc.vector.tensor_copy(av[0:65, :], op[0:65, :])
                rec = recpool.tile([1, 512], F32, tag="rec", name="rec")
                nc.vector.reciprocal_approx_fast(rec[:], op[0:1, :])
                rv = rvpool.tile([128, 512], F32, tag="rv", name="rv")
                nc.gpsimd.partition_broadcast(rv[:, :], rec[0:1, :])
                nc.sync.dma_start(dst, av[1:65, :])
                nc.vector.tensor_mul(
                    dst, dst, rv[half * 64:(half + 1) * 64, :])

        # ---- main schedule: chunk-major ----
        # F_0 emitted directly (nothing to interleave with yet)
        for j in range(4):
            for _ in gen_v(j):
                pass
        for hp in range(DT):
            for which in range(2):
                for _ in gen_qk(hp, 0, which):
                    pass

        for c in range(TC):
            if c + 1 < TC:
                for j in range(4 * (c + 1), 4 * (c + 1) + 4):
                    fillF.append(gen_v(j))
                for hp in range(DT):
                    for which in range(2):
                        fillF.append(gen_qk(hp, c + 1, which))
            for hp in range(DT):
                attention(hp, c)
            drain_all()
            fillP.append(gen_proj(c))
        drain_all()

    nc.compile()
    return nc


def _shard_inputs(x, Wk, bk, Wq, bq, Wv, bv, Wp, bp):
    bf = ml_dtypes.bfloat16
    in_maps = []
    for core in range(NCORES):
        b, hg = core // 2, core % 2
        sl = slice(hg * D, (hg + 1) * D)
        in_maps.append({
            "xT": np.ascontiguousarray(x[b].T).astype(bf),
            "wq": np.ascontiguousarray(Wq[:, sl]).astype(bf),
            "wk": np.ascontiguousarray(Wk[:, sl]).astype(bf),
            "wv": np.ascontiguousarray(Wv[:, sl]).astype(bf),
            "wp": np.ascontiguousarray(Wp[sl, :]).astype(bf),
            "bq": np.ascontiguousarray(
                bq[sl].reshape(DT, 128).T).astype(np.float32),
            "bk": np.ascontiguousarray(
                bk[sl].reshape(DT, 128).T).astype(np.float32),
            "bvbc": np.ascontiguousarray(
                np.broadcast_to(bv[sl], (128, D))).astype(np.float32),
        })
    return in_maps


def kernel(x, Wk, bk, Wq, bq, Wv, bv, Wp, bp, _trace=False, _trace_kwargs=None):
    x, Wk, bk, Wq, bq, Wv, bv, Wp, bp = [
        np.asarray(a) for a in (x, Wk, bk, Wq, bq, Wv, bv, Wp, bp)]
    if "nc" not in _cache:
        _cache["nc"] = _build()
    nc = _cache["nc"]
    in_maps = _shard_inputs(x, Wk, bk, Wq, bq, Wv, bv, Wp, bp)
    kw = dict(_trace_kwargs or {})
    res = run_bass_kernel_spmd(nc, in_maps, core_ids=list(range(NCORES)),
                               trace=_trace, **kw)
    out = np.empty((B, T, C), np.float32)
    for b in range(B):
        yp = res.results[2 * b]["ypT"] + res.results[2 * b + 1]["ypT"]
        out[b] = yp.T + bp[None, :]
    if _trace:
        _cache["last_results"] = res
    return out


# revision 12
# speedup vs baseline: 1.0887x; 1.0142x over previous
# Causal self-attention (B=4, T=2048, C=1024, 16 heads) on 8 NeuronCores.
# Sharding: core = (batch b = core//2) x (head-group hg = core%2, 8 heads each).
# Each core computes its 8 heads' attention for its batch plus the row-slice of
# the output projection; the host sums the two partial projections per batch.
#
# v3: chunk-major schedule with batched-1MB input DMAs on both HWDGE rings,
# valid-column-trimmed diagonal blocks (S/AV/exp skip fully-masked columns,
# masks shrink to one 128-col triangle strip), S one jt ahead of AV, and
# qkv/proj matmuls pumped into the ACT-bound attention stream as PE filler.
#
# Self-contained: hardcodes shapes; builds + compiles the Bass program once.

import contextlib

import numpy as np
import ml_dtypes

import concourse.bass as bass
import concourse.tile as tile
from concourse import bacc, mybir
from concourse.bass import AP
from concourse.bass_utils import run_bass_kernel_spmd

F32 = mybir.dt.float32
BF16 = mybir.dt.bfloat16
EXP = mybir.ActivationFunctionType.Exp
IDENT = mybir.ActivationFunctionType.Identity

B, T, C = 4, 2048, 1024
NH, HS = 16, 64
NHPC = 8          # heads per core
D = NHPC * HS     # 512: per-core qkv width
NCORES = 8
TT = T // 128     # 16 token tiles
TC = T // 512     # 4 token chunks
CT = C // 128     # 8 contraction tiles
DT = D // 128     # 4 d-tiles of qT/kT (= head pairs)
VW = 68           # per-head stride in v tile: [ones | v(64) | pad3]

_cache = {}


def _build():
    nc = bacc.Bacc("TRN2", target_bir_lowering=False, debug=False,
                   num_devices=NCORES)

    xT = nc.dram_tensor("xT", [C, T], BF16, kind="ExternalInput")
    wq = nc.dram_tensor("wq", [C, D], BF16, kind="ExternalInput")
    wk = nc.dram_tensor("wk", [C, D], BF16, kind="ExternalInput")
    wv = nc.dram_tensor("wv", [C, D], BF16, kind="ExternalInput")
    wp = nc.dram_tensor("wp", [D, C], BF16, kind="ExternalInput")
    bq = nc.dram_tensor("bq", [128, DT], F32, kind="ExternalInput")
    bk = nc.dram_tensor("bk", [128, DT], F32, kind="ExternalInput")
    bvbc = nc.dram_tensor("bvbc", [128, D], F32, kind="ExternalInput")
    ypT = nc.dram_tensor("ypT", [C, T], F32, kind="ExternalOutput")

    with tile.TileContext(nc) as tc, contextlib.ExitStack() as ctx:
        cpool = ctx.enter_context(tc.tile_pool(name="consts", bufs=1))
        xpool = ctx.enter_context(tc.tile_pool(name="x", bufs=1))
        wpool = ctx.enter_context(tc.tile_pool(name="w", bufs=1))
        qkpool = ctx.enter_context(tc.tile_pool(name="qk", bufs=1))
        vpool = ctx.enter_context(tc.tile_pool(name="v", bufs=1))
        opool = ctx.enter_context(tc.tile_pool(name="oT", bufs=1))
        ptpool = ctx.enter_context(tc.tile_pool(name="pt", bufs=8))
        ypool = ctx.enter_context(tc.tile_pool(name="y", bufs=4))
        rvpool = ctx.enter_context(tc.tile_pool(name="rv", bufs=4))
        recpool = ctx.enter_context(tc.tile_pool(name="rec", bufs=2))
        avpool = ctx.enter_context(tc.tile_pool(name="avb", bufs=4))
        qkv_ps = ctx.enter_context(
            tc.tile_pool(name="qkv_ps", bufs=2, space="PSUM"))
        s_ps = ctx.enter_context(
            tc.tile_pool(name="s_ps", bufs=2, space="PSUM"))
        o_ps = ctx.enter_context(
            tc.tile_pool(name="o_ps", bufs=2, space="PSUM"))

        # ---- merged weight/x tiles; batched ~1MB input DMAs, 2 rings ----
        wvw = wpool.tile([128, 128], BF16, tag="wvw", name="wvw")
        nc.sync.dma_start(wvw[:], wv.ap()[0:128, 0:128])  # tiny: warmup feed
        wvt = wpool.tile([128, CT * D], BF16, tag="wv", name="wv")
        wqt = wpool.tile([128, CT * D], BF16, tag="wq", name="wq")
        wkt = wpool.tile([128, CT * D], BF16, tag="wk", name="wk")
        wpt = wpool.tile([128, DT * C], BF16, tag="wp", name="wp")
        xt = wpool.tile([128, CT * T], BF16, tag="x", name="x")

        def wsrc(t):  # [C, D] dram -> [128, CT, D]
            return t.ap().rearrange("(ct p) d -> p ct d", p=128)

        xsrc = xT.ap().rearrange("(ct p) t -> p ct t", p=128)
        xre = xt[:].rearrange("p (ct t) -> p ct t", ct=CT)

        def dma_x_chunk(eng, cch):
            eng.dma_start(xre[:, :, cch * 512:(cch + 1) * 512],
                          xsrc[:, :, cch * 512:(cch + 1) * 512])

        nc.scalar.dma_start(
            wvt[:].rearrange("p (ct d) -> p ct d", ct=CT), wsrc(wv))
        dma_x_chunk(nc.sync, 0)
        nc.sync.dma_start(
            wqt[:].rearrange("p (ct d) -> p ct d", ct=CT), wsrc(wq))
        nc.scalar.dma_start(
            wkt[:].rearrange("p (ct d) -> p ct d", ct=CT), wsrc(wk))
        bqt = cpool.tile([128, DT], F32, tag="bq")
        bkt = cpool.tile([128, DT], F32, tag="bk")
        bvt = cpool.tile([128, D], F32, tag="bv")
        nc.sync.dma_start(bqt[:], bq.ap())
        nc.scalar.dma_start(bkt[:], bk.ap())
        nc.sync.dma_start(bvt[:], bvbc.ap())
        dma_x_chunk(nc.scalar, 1)
        dma_x_chunk(nc.sync, 2)
        nc.scalar.dma_start(
            wpt[:].rearrange("p (hp c) -> p hp c", hp=DT),
            wp.ap().rearrange("(hp p) c -> p hp c", p=128))
        dma_x_chunk(nc.sync, 3)

        def xap(ct, a, b):
            return xt[:, ct * T + a: ct * T + b]

        # ---- one-time consts ----
        ones8 = cpool.tile([128, NHPC], F32, tag="ones8")
        nc.vector.memset(ones8[:], 1.0)
        mf = cpool.tile([128, 128], F32, tag="mf", name="mf")
        nc.vector.memset(mf[:], 1.0)
        nc.gpsimd.affine_select(
            mf[:], mf[:], pattern=[[1, 128]],
            compare_op=mybir.AluOpType.is_ge, fill=0.0,
            base=0, channel_multiplier=-1)
        mstrip = cpool.tile([128, 128], BF16, tag="mstrip", name="mstrip")
        nc.vector.tensor_copy(mstrip[:], mf[:])

        # ---- warm-up matmuls on a tiny early tile: keep PE/HAM warm ----
        warm_ps = qkv_ps.tile([128, 512], F32, tag="qkv", name="warmps")
        for w in range(28):
            nc.tensor.matmul(
                warm_ps[:, 0:128], wvw[:], wvw[:],
                start=True, stop=True, skip_group_check=True)

        # ---- persistent tiles ----
        vt = [vpool.tile([128, NHPC * VW], BF16, tag=f"v{j}", name=f"v{j}")
              for j in range(TT)]
        qT = [qkpool.tile([128, T], BF16, tag=f"q{d}", name=f"q{d}")
              for d in range(DT)]
        kT = [qkpool.tile([128, T], BF16, tag=f"k{d}", name=f"k{d}")
              for d in range(DT)]
        oT = [opool.tile([128, T], BF16, tag=f"oT{d}", name=f"oT{d}")
              for d in range(DT)]

        # ---- filler generators: yield once per emitted matmul ----
        def gen_v(j):
            ps = qkv_ps.tile([128, D], F32, tag="qkv", name="qkvps")
            for ct in range(CT):
                nc.tensor.matmul(
                    ps[:], xap(ct, j * 128, (j + 1) * 128),
                    wvt[:, ct * D:(ct + 1) * D],
                    start=(ct == 0), stop=(ct == CT - 1))
                yield
            vre = vt[j][:].rearrange("p (h x) -> p h x", h=NHPC)
            nc.vector.tensor_copy(
                vre[:, :, 0:1], ones8[:].rearrange("p (h x) -> p h x", x=1))
            nc.vector.tensor_add(
                vre[:, :, 1:65],
                ps[:].rearrange("p (h x) -> p h x", h=NHPC),
                bvt[:].rearrange("p (h x) -> p h x", h=NHPC))

        def gen_qk(hp, c, which):
            wt_, bt_, out = ((wqt, bqt, qT), (wkt, bkt, kT))[which]
            ps = qkv_ps.tile([128, 512], F32, tag="qkv", name="qkvps")
            for ct in range(CT):
                nc.tensor.matmul(
                    ps[:], wt_[:, ct * D + hp * 128: ct * D + (hp + 1) * 128],
                    xap(ct, c * 512, (c + 1) * 512),
                    start=(ct == 0), stop=(ct == CT - 1))
                yield
            nc.scalar.activation(
                out[hp][:, c * 512:(c + 1) * 512], ps[:], IDENT,
                bias=bt_[:, hp:hp + 1])

        def gen_proj(c):
            for o in range(CT):
                ps = qkv_ps.tile([128, 512], F32, tag="qkv", name="qkvps")
                for hp in range(DT):
                    nc.tensor.matmul(
                        ps[:], wpt[:, hp * C + o * 128: hp * C + (o + 1) * 128],
                        oT[hp][:, c * 512:(c + 1) * 512],
                        start=(hp == 0), stop=(hp == DT - 1))
                    yield
                ys = ypool.tile([128, 512], F32, tag="y", name="ys")
                nc.vector.tensor_copy(ys[:], ps[:])
                nc.scalar.dma_start(
                    ypT.ap()[o * 128:(o + 1) * 128, c * 512:(c + 1) * 512],
                    ys[:])

        # two filler queues: F (projections for next chunk; must finish
        # before that chunk's attention) and P (output proj; can linger)
        fillF = []
        fillP = []

        def pump(n):
            for _ in range(n):
                q = fillF if fillF else fillP
                if not q:
                    return
                try:
                    next(q[0])
                except StopIteration:
                    q.pop(0)

        def drain_all():
            for q in (fillF, fillP):
                while q:
                    try:
                        next(q[0])
                    except StopIteration:
                        q.pop(0)

        # ---- attention block for head pair hp, query chunk c ----
        # Diagonal j-tiles (jt = 4c+t) skip their fully-masked first 128t
        # columns in S, exp and AV; only the 128-col triangle strip is masked.
        def attention(hp, c):
            njt = 4 * c + 4
            op0 = o_ps.tile([128, 512], F32, tag="o", name="ops")
            op1 = o_ps.tile([128, 512], F32, tag="o", name="ops")
            sps = {}

            def emit_S(jt):
                lo = 128 * (jt - 4 * c) if jt >= 4 * c else 0
                sp = s_ps.tile([128, 1024], F32, tag="s", name="sps")
                for half in range(2):
                    nc.tensor.matmul(
                        sp[:, half * 512 + lo:(half + 1) * 512],
                        kT[hp][half * 64:(half + 1) * 64,
                               jt * 128:(jt + 1) * 128],
                        qT[hp][half * 64:(half + 1) * 64,
                               c * 512 + lo:(c + 1) * 512],
                        start=True, stop=True)
                sps[jt] = sp

            emit_S(0)
            pcr = 0.0
            for jt in range(njt):
                if jt + 1 < njt:
                    emit_S(jt + 1)
                sp = sps.pop(jt)
                t = jt - 4 * c
                lo = 128 * t if t >= 0 else 0
                pt = ptpool.tile([128, 1024], BF16, tag="pt", name="pt")
                nc.scalar.activation(pt[:, lo:1024], sp[:, lo:1024],
                                     EXP, scale=0.125)
                if t >= 0:
                    nc.vector.tensor_mul(
                        pt[:, lo:lo + 128], pt[:, lo:lo + 128], mstrip[:])
                    nc.gpsimd.affine_select(
                        pt[:, 512 + lo:512 + lo + 128],
                        pt[:, 512 + lo:512 + lo + 128],
                        pattern=[[1, 128]],
                        compare_op=mybir.AluOpType.is_ge, fill=0.0,
                        base=0, channel_multiplier=-1)
                for half, op in ((0, op0), (1, op1)):
                    h = 2 * hp + half
                    nc.tensor.matmul(
                        op[0:65, lo:512], vt[jt][:, h * VW:h * VW + 65],
                        pt[:, half * 512 + lo:(half + 1) * 512],
                        start=(jt == 0), stop=(jt == njt - 1))
                pcr += 0.8
                if pcr >= 1.0:
                    k = int(pcr)
                    pump(k)
                    pcr -= k

            # normalization: stage unnormalized rows out of PSUM (releases
            # the bank), recip the rowsum straight from PSUM partition 0,
            # DMA-shift 1:65 -> oT, then scale oT in place (aligned).
            cs = slice(c * 512, (c + 1) * 512)
            for half, op, dst in ((0, op0, oT[hp][0:64, cs]),
                                  (1, op1, oT[hp][64:128, cs])):
                av = avpool.tile([128, 512], BF16, tag="avb", name="avb")
                nc.vector.tensor_copy(av[0:65, :], op[0:65, :])
                rec = recpool.tile([1, 512], F32, tag="rec", name="rec")
                nc.vector.reciprocal_approx_fast(rec[:], op[0:1, :])
                rv = rvpool.tile([128, 512], F32, tag="rv", name="rv")
                nc.gpsimd.partition_broadcast(rv[:, :], rec[0:1, :])
                nc.sync.dma_start(dst, av[1:65, :])
                nc.vector.tensor_mul(
                    dst, dst, rv[half * 64:(half + 1) * 64, :])

        # ---- main schedule: chunk-major ----
        # F_0 emitted directly (nothing to interleave with yet)
        for j in range(4):
            for _ in gen_v(j):
                pass
        for hp in range(DT):
            for which in range(2):
                for _ in gen_qk(hp, 0, which):
                    pass

        for c in range(TC):
            if c + 1 < TC:
                for j in range(4 * (c + 1), 4 * (c + 1) + 4):
                    fillF.append(gen_v(j))
                for hp in range(DT):
                    for which in range(2):
                        fillF.append(gen_qk(hp, c + 1, which))
            for hp in range(DT):
                attention(hp, c)
            drain_all()
            fillP.append(gen_proj(c))
        drain_all()

    nc.compile()
    return nc


def _shard_inputs(x, Wk, bk, Wq, bq, Wv, bv, Wp, bp):
    bf = ml_dtypes.bfloat16
    in_maps = []
    for core in range(NCORES):
        b, hg = core // 2, core % 2
        sl = slice(hg * D, (hg + 1) * D)
        in_maps.append({
            "xT": np.ascontiguousarray(x[b].T).astype(bf),
            "wq": np.ascontiguousarray(Wq[:, sl]).astype(bf),
            "wk": np.ascontiguousarray(Wk[:, sl]).astype(bf),
            "wv": np.ascontiguousarray(Wv[:, sl]).astype(bf),
            "wp": np.ascontiguousarray(Wp[sl, :]).astype(bf),
            "bq": np.ascontiguousarray(
                bq[sl].reshape(DT, 128).T).astype(np.float32),
            "bk": np.ascontiguousarray(
                bk[sl].reshape(DT, 128).T).astype(np.float32),
            "bvbc": np.ascontiguousarray(
                np.broadcast_to(bv[sl], (128, D))).astype(np.float32),
        })
    return in_maps


def kernel(x, Wk, bk, Wq, bq, Wv, bv, Wp, bp, _trace=False, _trace_kwargs=None):
    x, Wk, bk, Wq, bq, Wv, bv, Wp, bp = [
        np.asarray(a) for a in (x, Wk, bk, Wq, bq, Wv, bv, Wp, bp)]
    if "nc" not in _cache:
        _cache["nc"] = _build()
    nc = _cache["nc"]
    in_maps = _shard_inputs(x, Wk, bk, Wq, bq, Wv, bv, Wp, bp)
    kw = dict(_trace_kwargs or {})
    res = run_bass_kernel_spmd(nc, in_maps, core_ids=list(range(NCORES)),
                               trace=_trace, **kw)
    out = np.empty((B, T, C), np.float32)
    for b in range(B):
        yp = res.results[2 * b]["ypT"] + res.results[2 * b + 1]["ypT"]
        out[b] = yp.T + bp[None, :]
    if _trace:
        _cache["last_results"] = res
    return out


# revision 14
# speedup vs baseline: 1.1349x; 1.0425x over previous
# Causal self-attention (B=4, T=2048, C=1024, 16 heads) on 8 NeuronCores.
# Sharding: core = (batch b = core//2) x (head-group hg = core%2, 8 heads each).
# Each core computes its 8 heads' attention for its batch plus the row-slice of
# the output projection; the host sums the two partial projections per batch.
#
# v4: host packs every DRAM tensor so each DMA moves 8KB-contiguous runs per
# SBUF partition (128 descriptors per ~1MB transfer -> HBM-roofline input).
# Chunk-major schedule: per 512-token query chunk, next chunk's qkv and the
# previous chunk's output projection are pumped into the ACT-bound attention
# loop as PE filler; S runs one jt ahead of AV; diagonal blocks skip their
# fully-masked columns; output is staged bf16 and shipped once per chunk.
#
# Self-contained: hardcodes shapes; builds + compiles the Bass program once.

import contextlib

import numpy as np
import ml_dtypes

import concourse.bass as bass
import concourse.tile as tile
from concourse import bacc, mybir
from concourse.bass import AP
from concourse.bass_utils import run_bass_kernel_spmd

F32 = mybir.dt.float32
BF16 = mybir.dt.bfloat16
EXP = mybir.ActivationFunctionType.Exp
IDENT = mybir.ActivationFunctionType.Identity

B, T, C = 4, 2048, 1024
NH, HS = 16, 64
NHPC = 8          # heads per core
D = NHPC * HS     # 512: per-core qkv width
NCORES = 8
TT = T // 128     # 16 token tiles
TC = T // 512     # 4 token chunks
CT = C // 128     # 8 contraction tiles
DT = D // 128     # 4 d-tiles of qT/kT (= head pairs)
VW = 68           # per-head stride in v tile: [ones | v(64) | pad3]
XCH = CT * 512    # 4096: packed x/y columns per chunk

_cache = {}


def _build():
    nc = bacc.Bacc("TRN2", target_bir_lowering=False, debug=False,
                   num_devices=NCORES)

    # all inputs host-packed to [128, *] with per-partition-contiguous runs
    xP = nc.dram_tensor("xP", [128, TC * XCH], BF16, kind="ExternalInput")
    wqP = nc.dram_tensor("wqP", [128, CT * D], BF16, kind="ExternalInput")
    wkP = nc.dram_tensor("wkP", [128, CT * D], BF16, kind="ExternalInput")
    wvP = nc.dram_tensor("wvP", [128, CT * D], BF16, kind="ExternalInput")
    wpP = nc.dram_tensor("wpP", [128, DT * C], BF16, kind="ExternalInput")
    bq = nc.dram_tensor("bq", [128, DT], F32, kind="ExternalInput")
    bk = nc.dram_tensor("bk", [128, DT], F32, kind="ExternalInput")
    bvbc = nc.dram_tensor("bvbc", [128, D], F32, kind="ExternalInput")
    yP = nc.dram_tensor("yP", [128, TC * XCH], BF16, kind="ExternalOutput")

    with tile.TileContext(nc) as tc, contextlib.ExitStack() as ctx:
        cpool = ctx.enter_context(tc.tile_pool(name="consts", bufs=1))
        wpool = ctx.enter_context(tc.tile_pool(name="w", bufs=1))
        qkpool = ctx.enter_context(tc.tile_pool(name="qk", bufs=1))
        vpool = ctx.enter_context(tc.tile_pool(name="v", bufs=1))
        opool = ctx.enter_context(tc.tile_pool(name="oT", bufs=1))
        ptpool = ctx.enter_context(tc.tile_pool(name="pt", bufs=8))
        ypool = ctx.enter_context(tc.tile_pool(name="y", bufs=2))
        y1pool = ctx.enter_context(tc.tile_pool(name="y1", bufs=1))
        rvpool = ctx.enter_context(tc.tile_pool(name="rv", bufs=4))
        recpool = ctx.enter_context(tc.tile_pool(name="rec", bufs=2))
        avpool = ctx.enter_context(tc.tile_pool(name="avb", bufs=4))
        qkv_ps = ctx.enter_context(
            tc.tile_pool(name="qkv_ps", bufs=2, space="PSUM"))
        s_ps = ctx.enter_context(
            tc.tile_pool(name="s_ps", bufs=2, space="PSUM"))
        o_ps = ctx.enter_context(
            tc.tile_pool(name="o_ps", bufs=2, space="PSUM"))

        # ---- input DMAs: big contiguous transfers, both HWDGE rings ----
        wvw = wpool.tile([128, 128], BF16, tag="wvw", name="wvw")
        nc.sync.dma_start(wvw[:], wvP.ap()[:, 0:128])  # tiny: warmup feed
        wvt = wpool.tile([128, CT * D], BF16, tag="wv", name="wv")
        wqt = wpool.tile([128, CT * D], BF16, tag="wq", name="wq")
        wkt = wpool.tile([128, CT * D], BF16, tag="wk", name="wk")
        wpt = wpool.tile([128, DT * C], BF16, tag="wp", name="wp")
        xt = wpool.tile([128, TC * XCH], BF16, tag="x", name="x")

        nc.scalar.dma_start(wvt[:], wvP.ap())
        nc.sync.dma_start(xt[:, 0:XCH], xP.ap()[:, 0:XCH])
        nc.scalar.dma_start(wkt[:], wkP.ap())
        nc.sync.dma_start(wqt[:], wqP.ap())
        bqt = cpool.tile([128, DT], F32, tag="bq")
        bkt = cpool.tile([128, DT], F32, tag="bk")
        bvt = cpool.tile([128, D], F32, tag="bv")
        nc.sync.dma_start(bqt[:], bq.ap())
        nc.scalar.dma_start(bkt[:], bk.ap())
        nc.sync.dma_start(bvt[:], bvbc.ap())
        for cch, eng in ((1, nc.scalar), (2, nc.sync), (3, nc.scalar)):
            eng.dma_start(xt[:, cch * XCH:(cch + 1) * XCH],
                          xP.ap()[:, cch * XCH:(cch + 1) * XCH])
        nc.sync.dma_start(wpt[:], wpP.ap())

        def xap(ct, a, b):
            # x columns [a,b) of contraction tile ct; [a,b) within one chunk
            c = a // 512
            col = c * XCH + ct * 512 + (a - c * 512)
            return xt[:, col:col + (b - a)]

        # ---- one-time consts ----
        ones8 = cpool.tile([128, NHPC], F32, tag="ones8")
        nc.vector.memset(ones8[:], 1.0)
        mf = cpool.tile([128, 128], F32, tag="mf", name="mf")
        nc.vector.memset(mf[:], 1.0)
        nc.gpsimd.affine_select(
            mf[:], mf[:], pattern=[[1, 128]],
            compare_op=mybir.AluOpType.is_ge, fill=0.0,
            base=0, channel_multiplier=-1)
        mstrip = cpool.tile([128, 128], BF16, tag="mstrip", name="mstrip")
        nc.vector.tensor_copy(mstrip[:], mf[:])

        # ---- warm-up matmuls on a tiny early tile: keep PE/HAM warm ----
        warm_ps = qkv_ps.tile([128, 512], F32, tag="qkv", name="warmps")
        for w in range(28):
            nc.tensor.matmul(
                warm_ps[:, 0:128], wvw[:], wvw[:],
                start=True, stop=True, skip_group_check=True)

        # ---- persistent tiles ----
        vt = [vpool.tile([128, NHPC * VW], BF16, tag=f"v{j}", name=f"v{j}")
              for j in range(TT)]
        qT = [qkpool.tile([128, T], BF16, tag=f"q{d}", name=f"q{d}")
              for d in range(DT)]
        kT = [qkpool.tile([128, T], BF16, tag=f"k{d}", name=f"k{d}")
              for d in range(DT)]
        oT = [opool.tile([128, T], BF16, tag=f"oT{d}", name=f"oT{d}")
              for d in range(DT)]

        # ---- filler generators: yield once per emitted matmul ----
        def gen_v(j):
            ps = qkv_ps.tile([128, D], F32, tag="qkv", name="qkvps")
            for ct in range(CT):
                nc.tensor.matmul(
                    ps[:], xap(ct, j * 128, (j + 1) * 128),
                    wvt[:, ct * D:(ct + 1) * D],
                    start=(ct == 0), stop=(ct == CT - 1))
                yield
            vre = vt[j][:].rearrange("p (h x) -> p h x", h=NHPC)
            nc.vector.tensor_copy(
                vre[:, :, 0:1], ones8[:].rearrange("p (h x) -> p h x", x=1))
            nc.vector.tensor_add(
                vre[:, :, 1:65],
                ps[:].rearrange("p (h x) -> p h x", h=NHPC),
                bvt[:].rearrange("p (h x) -> p h x", h=NHPC))

        def gen_qk(hp, c, which):
            wt_, bt_, out = ((wqt, bqt, qT), (wkt, bkt, kT))[which]
            ps = qkv_ps.tile([128, 512], F32, tag="qkv", name="qkvps")
            for ct in range(CT):
                nc.tensor.matmul(
                    ps[:], wt_[:, ct * D + hp * 128: ct * D + (hp + 1) * 128],
                    xap(ct, c * 512, (c + 1) * 512),
                    start=(ct == 0), stop=(ct == CT - 1))
                yield
            nc.scalar.activation(
                out[hp][:, c * 512:(c + 1) * 512], ps[:], IDENT,
                bias=bt_[:, hp:hp + 1])

        def gen_proj(c, hps=(0, 1, 2, 3), ysrc=None):
            # proj over head pairs `hps`; adds ysrc (bf16 partial) if given;
            # ships the whole chunk in one DMA when it's the final pass.
            final = hps[-1] == DT - 1
            ys = ypool.tile([128, XCH], BF16, tag="y", name="ys") if final \
                else y1pool.tile([128, XCH], BF16, tag="y1", name="ys1")
            for o in range(CT):
                ps = qkv_ps.tile([128, 512], F32, tag="qkv", name="qkvps")
                for i, hp in enumerate(hps):
                    nc.tensor.matmul(
                        ps[:], wpt[:, hp * C + o * 128: hp * C + (o + 1) * 128],
                        oT[hp][:, c * 512:(c + 1) * 512],
                        start=(i == 0), stop=(i == len(hps) - 1))
                    yield
                osl = slice(o * 512, (o + 1) * 512)
                if ysrc is None:
                    nc.vector.tensor_copy(ys[:, osl], ps[:])
                else:
                    nc.vector.tensor_add(ys[:, osl], ps[:], ysrc[:, osl])
            if final:
                nc.scalar.dma_start(
                    yP.ap()[:, c * XCH:(c + 1) * XCH], ys[:])
            else:
                gen_proj.partial = ys

        # two filler queues: F (projections for next chunk; must finish
        # before that chunk's attention) and P (output proj; can linger)
        fillF = []
        fillP = []

        def pump(n):
            for _ in range(n):
                q = fillF if fillF else fillP
                if not q:
                    return
                try:
                    next(q[0])
                except StopIteration:
                    q.pop(0)

        def drain(q):
            while q:
                try:
                    next(q[0])
                except StopIteration:
                    q.pop(0)

        # ---- attention block for head pair hp, query chunk c ----
        # Diagonal j-tiles (jt = 4c+t) skip their fully-masked first 128t
        # columns in S, exp and AV; only the 128-col triangle strip is masked.
        def attention(hp, c, rate):
            njt = 4 * c + 4
            op0 = o_ps.tile([128, 512], F32, tag="o", name="ops")
            op1 = o_ps.tile([128, 512], F32, tag="o", name="ops")
            sps = {}

            def emit_S(jt):
                lo = 128 * (jt - 4 * c) if jt >= 4 * c else 0
                sp = s_ps.tile([128, 1024], F32, tag="s", name="sps")
                for half in range(2):
                    nc.tensor.matmul(
                        sp[:, half * 512 + lo:(half + 1) * 512],
                        kT[hp][half * 64:(half + 1) * 64,
                               jt * 128:(jt + 1) * 128],
                        qT[hp][half * 64:(half + 1) * 64,
                               c * 512 + lo:(c + 1) * 512],
                        start=True, stop=True)
                sps[jt] = sp

            emit_S(0)
            pcr = 0.0
            for jt in range(njt):
                if jt + 1 < njt:
                    emit_S(jt + 1)
                sp = sps.pop(jt)
                t = jt - 4 * c
                lo = 128 * t if t >= 0 else 0
                pt = ptpool.tile([128, 1024], BF16, tag="pt", name="pt")
                nc.scalar.activation(pt[:, lo:1024], sp[:, lo:1024],
                                     EXP, scale=0.125)
                if t >= 0:
                    nc.vector.tensor_mul(
                        pt[:, lo:lo + 128], pt[:, lo:lo + 128], mstrip[:])
                    nc.gpsimd.affine_select(
                        pt[:, 512 + lo:512 + lo + 128],
                        pt[:, 512 + lo:512 + lo + 128],
                        pattern=[[1, 128]],
                        compare_op=mybir.AluOpType.is_ge, fill=0.0,
                        base=0, channel_multiplier=-1)
                for half, op in ((0, op0), (1, op1)):
                    h = 2 * hp + half
                    nc.tensor.matmul(
                        op[0:65, lo:512], vt[jt][:, h * VW:h * VW + 65],
                        pt[:, half * 512 + lo:(half + 1) * 512],
                        start=(jt == 0), stop=(jt == njt - 1))
                pcr += rate
                if pcr >= 1.0:
                    k = int(pcr)
                    pump(k)
                    pcr -= k

            # normalization: stage unnormalized rows out of PSUM (releases
            # the bank), recip the rowsum straight from PSUM partition 0,
            # DMA-shift 1:65 -> oT, then scale oT in place (aligned).
            cs = slice(c * 512, (c + 1) * 512)
            for half, op, dst in ((0, op0, oT[hp][0:64, cs]),
                                  (1, op1, oT[hp][64:128, cs])):
                av = avpool.tile([128, 512], BF16, tag="avb", name="avb")
                nc.vector.tensor_copy(av[0:65, :], op[0:65, :])
                rec = recpool.tile([1, 512], F32, tag="rec", name="rec")
                nc.vector.reciprocal_approx_fast(rec[:], op[0:1, :])
                rv = rvpool.tile([128, 512], F32, tag="rv", name="rv")
                nc.gpsimd.partition_broadcast(rv[:, :], rec[0:1, :])
                nc.sync.dma_start(dst, av[1:65, :])
                nc.vector.tensor_mul(
                    dst, dst, rv[half * 64:(half + 1) * 64, :])

        # ---- main schedule: chunk-major ----
        # F_0 emitted directly (nothing to interleave with yet)
        for j in range(4):
            for _ in gen_v(j):
                pass
        for hp in range(DT):
            for which in range(2):
                for _ in gen_qk(hp, 0, which):
                    pass

        rates = [3.0, 1.2, 0.8, 1.2]
        for c in range(TC):
            if c + 1 < TC:
                for j in range(4 * (c + 1), 4 * (c + 1) + 4):
                    fillF.append(gen_v(j))
                for hp in range(DT):
                    for which in range(2):
                        fillF.append(gen_qk(hp, c + 1, which))
            for hp in range(DT):
                attention(hp, c, rates[c])
                if c == TC - 1 and hp == 1:
                    fillF.append(gen_proj(c, hps=(0, 1)))
            drain(fillF)
            if c != TC - 2:
                drain(fillP)          # finish old proj before queueing new
            if c < TC - 1:
                fillP.append(gen_proj(c))
        drain(fillP)                  # proj(2) remnants
        for _ in gen_proj(TC - 1, hps=(2, 3), ysrc=gen_proj.partial):
            pass

    nc.compile()
    return nc


def _shard_inputs(x, Wk, bk, Wq, bq, Wv, bv, Wp, bp):
    bf = ml_dtypes.bfloat16

    def packw(W, sl):  # [C, D-slice] -> [128, CT*D] partition-contiguous
        return np.ascontiguousarray(
            W[:, sl].reshape(CT, 128, D).transpose(1, 0, 2)
            .reshape(128, CT * D)).astype(bf)

    in_maps = []
    for core in range(NCORES):
        b, hg = core // 2, core % 2
        sl = slice(hg * D, (hg + 1) * D)
        xb = np.asarray(x[b], np.float32)  # [T, C]
        xp = (xb.T.reshape(CT, 128, TC, 512).transpose(1, 2, 0, 3)
              .reshape(128, TC * XCH))
        wpp = (Wp[sl, :].reshape(DT, 128, C).transpose(1, 0, 2)
               .reshape(128, DT * C))
        in_maps.append({
            "xP": np.ascontiguousarray(xp).astype(bf),
            "wqP": packw(Wq, sl),
            "wkP": packw(Wk, sl),
            "wvP": packw(Wv, sl),
            "wpP": np.ascontiguousarray(wpp).astype(bf),
            "bq": np.ascontiguousarray(
                bq[sl].reshape(DT, 128).T).astype(np.float32),
            "bk": np.ascontiguousarray(
                bk[sl].reshape(DT, 128).T).astype(np.float32),
            "bvbc": np.ascontiguousarray(
                np.broadcast_to(bv[sl], (128, D))).astype(np.float32),
        })
    return in_maps


def kernel(x, Wk, bk, Wq, bq, Wv, bv, Wp, bp, _trace=False, _trace_kwargs=None):
    x, Wk, bk, Wq, bq, Wv, bv, Wp, bp = [
        np.asarray(a) for a in (x, Wk, bk, Wq, bq, Wv, bv, Wp, bp)]
    if "nc" not in _cache:
        _cache["nc"] = _build()
    nc = _cache["nc"]
    in_maps = _shard_inputs(x, Wk, bk, Wq, bq, Wv, bv, Wp, bp)
    kw = dict(_trace_kwargs or {})
    res = run_bass_kernel_spmd(nc, in_maps, core_ids=list(range(NCORES)),
                               trace=_trace, **kw)
    out = np.empty((B, T, C), np.float32)
    for b in range(B):
        yp = (res.results[2 * b]["yP"].astype(np.float32)
              + res.results[2 * b + 1]["yP"].astype(np.float32))
        # yP[p, c*XCH + o*512 + d] = y_partial[o*128+p, c*512+d]
        yp = (yp.reshape(128, TC, CT, 512).transpose(2, 0, 1, 3)
              .reshape(C, T))
        out[b] = yp.T + bp[None, :]
    if _trace:
        _cache["last_results"] = res
    return out


# revision 19
# speedup vs baseline: 1.1487x; 1.0122x over previous
# Causal self-attention (B=4, T=2048, C=1024, 16 heads) on 8 NeuronCores.
# Sharding: core = (batch b = core//2) x (head-group hg = core%2, 8 heads each).
# Each core computes its 8 heads' attention for its batch plus the row-slice of
# the output projection; the host sums the two partial projections per batch.
#
# v4: host packs every DRAM tensor so each DMA moves 8KB-contiguous runs per
# SBUF partition (128 descriptors per ~1MB transfer -> HBM-roofline input).
# Chunk-major schedule: per 512-token query chunk, next chunk's qkv and the
# previous chunk's output projection are pumped into the ACT-bound attention
# loop as PE filler; S runs one jt ahead of AV; diagonal blocks skip their
# fully-masked columns; output is staged bf16 and shipped once per chunk.
#
# Self-contained: hardcodes shapes; builds + compiles the Bass program once.

import contextlib

import numpy as np
import ml_dtypes

import concourse.bass as bass
import concourse.tile as tile
from concourse import bacc, mybir
from concourse.bass import AP
from concourse.bass_utils import run_bass_kernel_spmd

F32 = mybir.dt.float32
BF16 = mybir.dt.bfloat16
EXP = mybir.ActivationFunctionType.Exp
IDENT = mybir.ActivationFunctionType.Identity

B, T, C = 4, 2048, 1024
NH, HS = 16, 64
NHPC = 8          # heads per core
D = NHPC * HS     # 512: per-core qkv width
NCORES = 8
TT = T // 128     # 16 token tiles
TC = T // 512     # 4 token chunks
CT = C // 128     # 8 contraction tiles
DT = D // 128     # 4 d-tiles of qT/kT (= head pairs)
VW = 68           # per-head stride in v tile: [ones | v(64) | pad3]
XCH = CT * 512    # 4096: packed x/y columns per chunk

_cache = {}


def _build():
    nc = bacc.Bacc("TRN2", target_bir_lowering=False, debug=False,
                   num_devices=NCORES)

    # all inputs host-packed to [128, *] with per-partition-contiguous runs
    xP = nc.dram_tensor("xP", [128, TC * XCH], BF16, kind="ExternalInput")
    wqP = nc.dram_tensor("wqP", [128, CT * D], BF16, kind="ExternalInput")
    wkP = nc.dram_tensor("wkP", [128, CT * D], BF16, kind="ExternalInput")
    wvP = nc.dram_tensor("wvP", [128, CT * D], BF16, kind="ExternalInput")
    wpP = nc.dram_tensor("wpP", [128, DT * C], BF16, kind="ExternalInput")
    bq = nc.dram_tensor("bq", [128, DT], F32, kind="ExternalInput")
    bk = nc.dram_tensor("bk", [128, DT], F32, kind="ExternalInput")
    bv1 = nc.dram_tensor("bv1", [1, D], F32, kind="ExternalInput")
    yP = nc.dram_tensor("yP", [128, TC * XCH], BF16, kind="ExternalOutput")

    with tile.TileContext(nc) as tc, contextlib.ExitStack() as ctx:
        cpool = ctx.enter_context(tc.tile_pool(name="consts", bufs=1))
        wpool = ctx.enter_context(tc.tile_pool(name="w", bufs=1))
        qkpool = ctx.enter_context(tc.tile_pool(name="qk", bufs=1))
        vpool = ctx.enter_context(tc.tile_pool(name="v", bufs=1))
        opool = ctx.enter_context(tc.tile_pool(name="oT", bufs=1))
        ptpool = ctx.enter_context(tc.tile_pool(name="pt", bufs=8))
        ypool = ctx.enter_context(tc.tile_pool(name="y", bufs=2))
        y1pool = ctx.enter_context(tc.tile_pool(name="y1", bufs=1))
        rvpool = ctx.enter_context(tc.tile_pool(name="rv", bufs=4))
        recpool = ctx.enter_context(tc.tile_pool(name="rec", bufs=2))
        avpool = ctx.enter_context(tc.tile_pool(name="avb", bufs=4))
        qkv_ps = ctx.enter_context(
            tc.tile_pool(name="qkv_ps", bufs=2, space="PSUM"))
        s_ps = ctx.enter_context(
            tc.tile_pool(name="s_ps", bufs=2, space="PSUM"))
        o_ps = ctx.enter_context(
            tc.tile_pool(name="o_ps", bufs=2, space="PSUM"))

        # ---- input DMAs: big contiguous transfers, both HWDGE rings ----
        wvw = wpool.tile([128, 128], BF16, tag="wvw", name="wvw")
        nc.sync.dma_start(wvw[:], wvP.ap()[:, 0:128])  # tiny: warmup feed
        wvt = wpool.tile([128, CT * D], BF16, tag="wv", name="wv")
        wqt = wpool.tile([128, CT * D], BF16, tag="wq", name="wq")
        wkt = wpool.tile([128, CT * D], BF16, tag="wk", name="wk")
        wpt = wpool.tile([128, DT * C], BF16, tag="wp", name="wp")
        xt = wpool.tile([128, TC * XCH], BF16, tag="x", name="x")

        # halves so consumers start after ~0.5MB; sync feeds v/q + x1/x2,
        # scalar feeds wv/wk + x3/wp.  bv arrives tiny and is broadcast here.
        HW2 = CT * D // 2
        nc.scalar.dma_start(wvt[:, 0:HW2], wvP.ap()[:, 0:HW2])
        nc.scalar.dma_start(wvt[:, HW2:], wvP.ap()[:, HW2:])
        nc.sync.dma_start(xt[:, 0:XCH // 2], xP.ap()[:, 0:XCH // 2])
        nc.sync.dma_start(xt[:, XCH // 2:XCH], xP.ap()[:, XCH // 2:XCH])
        nc.scalar.dma_start(wkt[:, 0:HW2], wkP.ap()[:, 0:HW2])
        nc.scalar.dma_start(wkt[:, HW2:], wkP.ap()[:, HW2:])
        nc.sync.dma_start(wqt[:, 0:HW2], wqP.ap()[:, 0:HW2])
        nc.sync.dma_start(wqt[:, HW2:], wqP.ap()[:, HW2:])
        bqt = cpool.tile([128, DT], F32, tag="bq")
        bkt = cpool.tile([128, DT], F32, tag="bk")
        bvs = cpool.tile([1, D], F32, tag="bvs")
        bvt = cpool.tile([128, D], F32, tag="bv")
        nc.sync.dma_start(bqt[:], bq.ap())
        nc.scalar.dma_start(bkt[:], bk.ap())
        nc.scalar.dma_start(bvs[:], bv1.ap())
        nc.gpsimd.partition_broadcast(bvt[:, :], bvs[0:1, :])
        for cch, eng in ((1, nc.sync), (2, nc.sync), (3, nc.scalar)):
            eng.dma_start(xt[:, cch * XCH:(cch + 1) * XCH],
                          xP.ap()[:, cch * XCH:(cch + 1) * XCH])
        nc.scalar.dma_start(wpt[:], wpP.ap())

        def xap(ct, a, b):
            # x columns [a,b) of contraction tile ct; [a,b) within one chunk
            c = a // 512
            col = c * XCH + ct * 512 + (a - c * 512)
            return xt[:, col:col + (b - a)]

        # ---- one-time consts ----
        ones8 = cpool.tile([128, NHPC], F32, tag="ones8")
        nc.vector.memset(ones8[:], 1.0)
        mf = cpool.tile([128, 128], F32, tag="mf", name="mf")
        nc.vector.memset(mf[:], 1.0)
        nc.gpsimd.affine_select(
            mf[:], mf[:], pattern=[[1, 128]],
            compare_op=mybir.AluOpType.is_ge, fill=0.0,
            base=0, channel_multiplier=-1)
        mstrip = cpool.tile([128, 128], BF16, tag="mstrip", name="mstrip")
        nc.vector.tensor_copy(mstrip[:], mf[:])

        # ---- warm-up matmuls on a tiny early tile: keep PE/HAM warm ----
        warm_ps = qkv_ps.tile([128, 512], F32, tag="qkv", name="warmps")
        for w in range(28):
            nc.tensor.matmul(
                warm_ps[:, 0:128], wvw[:], wvw[:],
                start=True, stop=True, skip_group_check=True)

        # ---- persistent tiles ----
        vt = [vpool.tile([128, NHPC * VW], BF16, tag=f"v{j}", name=f"v{j}")
              for j in range(TT)]
        qT = [qkpool.tile([128, T], BF16, tag=f"q{d}", name=f"q{d}")
              for d in range(DT)]
        kT = [qkpool.tile([128, T], BF16, tag=f"k{d}", name=f"k{d}")
              for d in range(DT)]
        oT = [opool.tile([128, T], BF16, tag=f"oT{d}", name=f"oT{d}")
              for d in range(DT)]

        # ---- filler generators: yield once per emitted matmul ----
        def gen_v(j):
            ps = qkv_ps.tile([128, D], F32, tag="qkv", name="qkvps")
            for ct in range(CT):
                nc.tensor.matmul(
                    ps[:], xap(ct, j * 128, (j + 1) * 128),
                    wvt[:, ct * D:(ct + 1) * D],
                    start=(ct == 0), stop=(ct == CT - 1))
                yield
            vre = vt[j][:].rearrange("p (h x) -> p h x", h=NHPC)
            nc.vector.tensor_copy(
                vre[:, :, 0:1], ones8[:].rearrange("p (h x) -> p h x", x=1))
            nc.vector.tensor_add(
                vre[:, :, 1:65],
                ps[:].rearrange("p (h x) -> p h x", h=NHPC),
                bvt[:].rearrange("p (h x) -> p h x", h=NHPC))

        def gen_qk(hp, c, which):
            wt_, bt_, out = ((wqt, bqt, qT), (wkt, bkt, kT))[which]
            ps = qkv_ps.tile([128, 512], F32, tag="qkv", name="qkvps")
            for ct in range(CT):
                nc.tensor.matmul(
                    ps[:], wt_[:, ct * D + hp * 128: ct * D + (hp + 1) * 128],
                    xap(ct, c * 512, (c + 1) * 512),
                    start=(ct == 0), stop=(ct == CT - 1))
                yield
            nc.vector.tensor_scalar_add(
                out[hp][:, c * 512:(c + 1) * 512], ps[:], bt_[:, hp:hp + 1])

        def gen_proj(c, hps=(0, 1, 2, 3), ysrc=None):
            # proj over head pairs `hps`; adds ysrc (bf16 partial) if given;
            # ships the whole chunk in one DMA when it's the final pass.
            final = hps[-1] == DT - 1
            ys = ypool.tile([128, XCH], BF16, tag="y", name="ys") if final \
                else y1pool.tile([128, XCH], BF16, tag="y1", name="ys1")
            for o in range(CT):
                ps = qkv_ps.tile([128, 512], F32, tag="qkv", name="qkvps")
                for i, hp in enumerate(hps):
                    nc.tensor.matmul(
                        ps[:], wpt[:, hp * C + o * 128: hp * C + (o + 1) * 128],
                        oT[hp][:, c * 512:(c + 1) * 512],
                        start=(i == 0), stop=(i == len(hps) - 1))
                    yield
                osl = slice(o * 512, (o + 1) * 512)
                if ysrc is None:
                    nc.vector.tensor_copy(ys[:, osl], ps[:])
                else:
                    nc.vector.tensor_add(ys[:, osl], ps[:], ysrc[:, osl])
                if final and o == CT // 2 - 1:
                    nc.scalar.dma_start(
                        yP.ap()[:, c * XCH:c * XCH + XCH // 2],
                        ys[:, 0:XCH // 2])
            if final:
                nc.scalar.dma_start(
                    yP.ap()[:, c * XCH + XCH // 2:(c + 1) * XCH],
                    ys[:, XCH // 2:])
            else:
                gen_proj.partial = ys

        # two filler queues: F (projections for next chunk; must finish
        # before that chunk's attention) and P (output proj; can linger)
        fillF = []
        fillP = []

        def pump(n):
            for _ in range(n):
                q = fillF if fillF else fillP
                if not q:
                    return
                try:
                    next(q[0])
                except StopIteration:
                    q.pop(0)

        def drain(q):
            while q:
                try:
                    next(q[0])
                except StopIteration:
                    q.pop(0)

        # ---- attention block for head pair hp, query chunk c ----
        # Diagonal j-tiles (jt = 4c+t) skip their fully-masked first 128t
        # columns in S, exp and AV; only the 128-col triangle strip is masked.
        def attention(hp, c, rate):
            njt = 4 * c + 4
            op0 = o_ps.tile([128, 512], F32, tag="o", name="ops")
            op1 = o_ps.tile([128, 512], F32, tag="o", name="ops")
            sps = {}

            def emit_S(jt):
                lo = 128 * (jt - 4 * c) if jt >= 4 * c else 0
                sp = s_ps.tile([128, 1024], F32, tag="s", name="sps")
                for half in range(2):
                    nc.tensor.matmul(
                        sp[:, half * 512 + lo:(half + 1) * 512],
                        kT[hp][half * 64:(half + 1) * 64,
                               jt * 128:(jt + 1) * 128],
                        qT[hp][half * 64:(half + 1) * 64,
                               c * 512 + lo:(c + 1) * 512],
                        start=True, stop=True)
                sps[jt] = sp

            emit_S(0)
            pcr = 0.0
            for jt in range(njt):
                if jt + 1 < njt:
                    emit_S(jt + 1)
                sp = sps.pop(jt)
                t = jt - 4 * c
                lo = 128 * t if t >= 0 else 0
                pt = ptpool.tile([128, 1024], BF16, tag="pt", name="pt")
                nc.scalar.activation(pt[:, lo:1024], sp[:, lo:1024],
                                     EXP, scale=0.125)
                if t >= 0:
                    nc.vector.tensor_mul(
                        pt[:, lo:lo + 128], pt[:, lo:lo + 128], mstrip[:])
                    nc.gpsimd.affine_select(
                        pt[:, 512 + lo:512 + lo + 128],
                        pt[:, 512 + lo:512 + lo + 128],
                        pattern=[[1, 128]],
                        compare_op=mybir.AluOpType.is_ge, fill=0.0,
                        base=0, channel_multiplier=-1)
                for half, op in ((0, op0), (1, op1)):
                    h = 2 * hp + half
                    nc.tensor.matmul(
                        op[0:65, lo:512], vt[jt][:, h * VW:h * VW + 65],
                        pt[:, half * 512 + lo:(half + 1) * 512],
                        start=(jt == 0), stop=(jt == njt - 1))
                pcr += rate
                if pcr >= 1.0:
                    k = int(pcr)
                    pump(k)
                    pcr -= k

            # normalization: stage unnormalized rows out of PSUM (releases
            # the bank), recip the rowsum straight from PSUM partition 0,
            # DMA-shift 1:65 -> oT, then scale oT in place (aligned).
            cs = slice(c * 512, (c + 1) * 512)
            for half, op, dst in ((0, op0, oT[hp][0:64, cs]),
                                  (1, op1, oT[hp][64:128, cs])):
                av = avpool.tile([128, 512], BF16, tag="avb", name="avb")
                nc.vector.tensor_copy(av[0:65, :], op[0:65, :])
                rec = recpool.tile([1, 512], F32, tag="rec", name="rec")
                nc.vector.reciprocal_approx_fast(rec[:], op[0:1, :])
                rv = rvpool.tile([128, 512], F32, tag="rv", name="rv")
                nc.gpsimd.partition_broadcast(rv[:, :], rec[0:1, :])
                nc.sync.dma_start(dst, av[1:65, :])
                nc.vector.tensor_mul(
                    dst, dst, rv[half * 64:(half + 1) * 64, :])

        # ---- main schedule: chunk-major ----
        # F_0 emitted directly (nothing to interleave with yet)
        for j in range(4):
            for _ in gen_v(j):
                pass
        for hp in range(DT):
            for which in range(2):
                for _ in gen_qk(hp, 0, which):
                    pass

        rates = [3.0, 1.2, 0.8, 1.2]
        for c in range(TC):
            if c + 1 < TC:
                for j in range(4 * (c + 1), 4 * (c + 1) + 4):
                    fillF.append(gen_v(j))
                for hp in range(DT):
                    for which in range(2):
                        fillF.append(gen_qk(hp, c + 1, which))
            for hp in range(DT):
                attention(hp, c, rates[c])
                if c == TC - 1 and hp == 1:
                    fillF.append(gen_proj(c, hps=(0, 1)))
            drain(fillF)
            if c != TC - 2:
                drain(fillP)          # finish old proj before queueing new
            if c < TC - 1:
                fillP.append(gen_proj(c))
        drain(fillP)                  # proj(2) remnants
        for _ in gen_proj(TC - 1, hps=(2, 3), ysrc=gen_proj.partial):
            pass

    nc.compile()
    return nc


def _shard_inputs(x, Wk, bk, Wq, bq, Wv, bv, Wp, bp):
    bf = ml_dtypes.bfloat16

    def packw(W, sl):  # [C, D-slice] -> [128, CT*D] partition-contiguous
        return np.ascontiguousarray(
            W[:, sl].reshape(CT, 128, D).transpose(1, 0, 2)
            .reshape(128, CT * D)).astype(bf)

    in_maps = []
    for core in range(NCORES):
        b, hg = core // 2, core % 2
        sl = slice(hg * D, (hg + 1) * D)
        xb = np.asarray(x[b], np.float32)  # [T, C]
        xp = (xb.T.reshape(CT, 128, TC, 512).transpose(1, 2, 0, 3)
              .reshape(128, TC * XCH))
        wpp = (Wp[sl, :].reshape(DT, 128, C).transpose(1, 0, 2)
               .reshape(128, DT * C))
        in_maps.append({
            "xP": np.ascontiguousarray(xp).astype(bf),
            "wqP": packw(Wq, sl),
            "wkP": packw(Wk, sl),
            "wvP": packw(Wv, sl),
            "wpP": np.ascontiguousarray(wpp).astype(bf),
            "bq": np.ascontiguousarray(
                bq[sl].reshape(DT, 128).T).astype(np.float32),
            "bk": np.ascontiguousarray(
                bk[sl].reshape(DT, 128).T).astype(np.float32),
            "bv1": np.ascontiguousarray(
                bv[sl].reshape(1, D)).astype(np.float32),
        })
    return in_maps


def kernel(x, Wk, bk, Wq, bq, Wv, bv, Wp, bp, _trace=False, _trace_kwargs=None):
    x, Wk, bk, Wq, bq, Wv, bv, Wp, bp = [
        np.asarray(a) for a in (x, Wk, bk, Wq, bq, Wv, bv, Wp, bp)]
    if "nc" not in _cache:
        _cache["nc"] = _build()
    nc = _cache["nc"]
    in_maps = _shard_inputs(x, Wk, bk, Wq, bq, Wv, bv, Wp, bp)
    kw = dict(_trace_kwargs or {})
    res = run_bass_kernel_spmd(nc, in_maps, core_ids=list(range(NCORES)),
                               trace=_trace, **kw)
    out = np.empty((B, T, C), np.float32)
    for b in range(B):
        yp = (res.results[2 * b]["yP"].astype(np.float32)
              + res.results[2 * b + 1]["yP"].astype(np.float32))
        # yP[p, c*XCH + o*512 + d] = y_partial[o*128+p, c*512+d]
        yp = (yp.reshape(128, TC, CT, 512).transpose(2, 0, 1, 3)
              .reshape(C, T))
        out[b] = yp.T + bp[None, :]
    if _trace:
        _cache["last_results"] = res
    return out


# revision 21
# speedup vs baseline: 1.1493x; 1.0005x over previous
# Causal self-attention (B=4, T=2048, C=1024, 16 heads) on 8 NeuronCores.
# Sharding: core = (batch b = core//2) x (head-group hg = core%2, 8 heads each).
# Each core computes its 8 heads' attention for its batch plus the row-slice of
# the output projection; the host sums the two partial projections per batch.
#
# v4: host packs every DRAM tensor so each DMA moves 8KB-contiguous runs per
# SBUF partition (128 descriptors per ~1MB transfer -> HBM-roofline input).
# Chunk-major schedule: per 512-token query chunk, next chunk's qkv and the
# previous chunk's output projection are pumped into the ACT-bound attention
# loop as PE filler; S runs one jt ahead of AV; diagonal blocks skip their
# fully-masked columns; output is staged bf16 and shipped once per chunk.
#
# Self-contained: hardcodes shapes; builds + compiles the Bass program once.

import contextlib

import numpy as np
import ml_dtypes

import concourse.bass as bass
import concourse.tile as tile
from concourse import bacc, mybir
from concourse.bass import AP
from concourse.bass_utils import run_bass_kernel_spmd

F32 = mybir.dt.float32
BF16 = mybir.dt.bfloat16
EXP = mybir.ActivationFunctionType.Exp
IDENT = mybir.ActivationFunctionType.Identity

B, T, C = 4, 2048, 1024
NH, HS = 16, 64
NHPC = 8          # heads per core
D = NHPC * HS     # 512: per-core qkv width
NCORES = 8
TT = T // 128     # 16 token tiles
TC = T // 512     # 4 token chunks
CT = C // 128     # 8 contraction tiles
DT = D // 128     # 4 d-tiles of qT/kT (= head pairs)
VW = 68           # per-head stride in v tile: [ones | v(64) | pad3]
XCH = CT * 512    # 4096: packed x/y columns per chunk

_cache = {}


def _build():
    nc = bacc.Bacc("TRN2", target_bir_lowering=False, debug=False,
                   num_devices=NCORES)

    # all inputs host-packed to [128, *] with per-partition-contiguous runs
    xP = nc.dram_tensor("xP", [128, TC * XCH], BF16, kind="ExternalInput")
    wqP = nc.dram_tensor("wqP", [128, CT * D], BF16, kind="ExternalInput")
    wkP = nc.dram_tensor("wkP", [128, CT * D], BF16, kind="ExternalInput")
    wvP = nc.dram_tensor("wvP", [128, CT * D], BF16, kind="ExternalInput")
    wpP = nc.dram_tensor("wpP", [128, DT * C], BF16, kind="ExternalInput")
    bq = nc.dram_tensor("bq", [128, DT], F32, kind="ExternalInput")
    bk = nc.dram_tensor("bk", [128, DT], F32, kind="ExternalInput")
    bv1 = nc.dram_tensor("bv1", [1, D], F32, kind="ExternalInput")
    yP = nc.dram_tensor("yP", [128, TC * XCH], BF16, kind="ExternalOutput")

    with tile.TileContext(nc) as tc, contextlib.ExitStack() as ctx:
        cpool = ctx.enter_context(tc.tile_pool(name="consts", bufs=1))
        wpool = ctx.enter_context(tc.tile_pool(name="w", bufs=1))
        qkpool = ctx.enter_context(tc.tile_pool(name="qk", bufs=1))
        vpool = ctx.enter_context(tc.tile_pool(name="v", bufs=1))
        opool = ctx.enter_context(tc.tile_pool(name="oT", bufs=1))
        ptpool = ctx.enter_context(tc.tile_pool(name="pt", bufs=8))
        ypool = ctx.enter_context(tc.tile_pool(name="y", bufs=2))
        y1pool = ctx.enter_context(tc.tile_pool(name="y1", bufs=1))
        rvpool = ctx.enter_context(tc.tile_pool(name="rv", bufs=4))
        recpool = ctx.enter_context(tc.tile_pool(name="rec", bufs=2))
        avpool = ctx.enter_context(tc.tile_pool(name="avb", bufs=4))
        qkv_ps = ctx.enter_context(
            tc.tile_pool(name="qkv_ps", bufs=2, space="PSUM"))
        s_ps = ctx.enter_context(
            tc.tile_pool(name="s_ps", bufs=2, space="PSUM"))
        o_ps = ctx.enter_context(
            tc.tile_pool(name="o_ps", bufs=2, space="PSUM"))

        # ---- input DMAs: big contiguous transfers, both HWDGE rings ----
        wvw = wpool.tile([128, 128], BF16, tag="wvw", name="wvw")
        nc.sync.dma_start(wvw[:], wvP.ap()[:, 0:128])  # tiny: warmup feed
        wvt = wpool.tile([128, CT * D], BF16, tag="wv", name="wv")
        wqt = wpool.tile([128, CT * D], BF16, tag="wq", name="wq")
        wkt = wpool.tile([128, CT * D], BF16, tag="wk", name="wk")
        wpt = wpool.tile([128, DT * C], BF16, tag="wp", name="wp")
        xt = wpool.tile([128, TC * XCH], BF16, tag="x", name="x")

        # quarters so consumers start after ~0.25MB; sync feeds x0/wq + x1/x2,
        # scalar feeds wv/wk + x3/wp.  bv arrives tiny and is broadcast here.
        HW4 = CT * D // 4
        for i in range(4):
            nc.scalar.dma_start(wvt[:, i * HW4:(i + 1) * HW4],
                                wvP.ap()[:, i * HW4:(i + 1) * HW4])
            nc.sync.dma_start(xt[:, i * HW4:(i + 1) * HW4],
                              xP.ap()[:, i * HW4:(i + 1) * HW4])
        for i in range(4):
            nc.scalar.dma_start(wkt[:, i * HW4:(i + 1) * HW4],
                                wkP.ap()[:, i * HW4:(i + 1) * HW4])
            nc.sync.dma_start(wqt[:, i * HW4:(i + 1) * HW4],
                              wqP.ap()[:, i * HW4:(i + 1) * HW4])
        bqt = cpool.tile([128, DT], F32, tag="bq")
        bkt = cpool.tile([128, DT], F32, tag="bk")
        bvs = cpool.tile([1, D], F32, tag="bvs")
        bvt = cpool.tile([128, D], F32, tag="bv")
        nc.sync.dma_start(bqt[:], bq.ap())
        nc.scalar.dma_start(bkt[:], bk.ap())
        nc.scalar.dma_start(bvs[:], bv1.ap())
        nc.gpsimd.partition_broadcast(bvt[:, :], bvs[0:1, :])
        for cch, eng in ((1, nc.sync), (2, nc.sync), (3, nc.scalar)):
            eng.dma_start(xt[:, cch * XCH:(cch + 1) * XCH],
                          xP.ap()[:, cch * XCH:(cch + 1) * XCH])
        nc.scalar.dma_start(wpt[:], wpP.ap())

        def xap(ct, a, b):
            # x columns [a,b) of contraction tile ct; [a,b) within one chunk
            c = a // 512
            col = c * XCH + ct * 512 + (a - c * 512)
            return xt[:, col:col + (b - a)]

        # ---- one-time consts ----
        ones8 = cpool.tile([128, NHPC], F32, tag="ones8")
        nc.vector.memset(ones8[:], 1.0)
        mf = cpool.tile([128, 128], F32, tag="mf", name="mf")
        nc.vector.memset(mf[:], 1.0)
        nc.gpsimd.affine_select(
            mf[:], mf[:], pattern=[[1, 128]],
            compare_op=mybir.AluOpType.is_ge, fill=0.0,
            base=0, channel_multiplier=-1)
        mstrip = cpool.tile([128, 128], BF16, tag="mstrip", name="mstrip")
        nc.vector.tensor_copy(mstrip[:], mf[:])

        # ---- warm-up matmuls on a tiny early tile: keep PE/HAM warm ----
        warm_ps = qkv_ps.tile([128, 512], F32, tag="qkv", name="warmps")
        for w in range(28):
            nc.tensor.matmul(
                warm_ps[:, 0:128], wvw[:], wvw[:],
                start=True, stop=True, skip_group_check=True)

        # ---- persistent tiles ----
        vt = [vpool.tile([128, NHPC * VW], BF16, tag=f"v{j}", name=f"v{j}")
              for j in range(TT)]
        qT = [qkpool.tile([128, T], BF16, tag=f"q{d}", name=f"q{d}")
              for d in range(DT)]
        kT = [qkpool.tile([128, T], BF16, tag=f"k{d}", name=f"k{d}")
              for d in range(DT)]
        oT = [opool.tile([128, T], BF16, tag=f"oT{d}", name=f"oT{d}")
              for d in range(DT)]

        # ---- filler generators: yield once per emitted matmul ----
        def gen_v(j):
            ps = qkv_ps.tile([128, D], F32, tag="qkv", name="qkvps")
            for ct in range(CT):
                nc.tensor.matmul(
                    ps[:], xap(ct, j * 128, (j + 1) * 128),
                    wvt[:, ct * D:(ct + 1) * D],
                    start=(ct == 0), stop=(ct == CT - 1))
                yield
            vre = vt[j][:].rearrange("p (h x) -> p h x", h=NHPC)
            nc.vector.tensor_copy(
                vre[:, :, 0:1], ones8[:].rearrange("p (h x) -> p h x", x=1))
            nc.vector.tensor_add(
                vre[:, :, 1:65],
                ps[:].rearrange("p (h x) -> p h x", h=NHPC),
                bvt[:].rearrange("p (h x) -> p h x", h=NHPC))

        def gen_qk(hp, c, which):
            wt_, bt_, out = ((wqt, bqt, qT), (wkt, bkt, kT))[which]
            ps = qkv_ps.tile([128, 512], F32, tag="qkv", name="qkvps")
            for ct in range(CT):
                nc.tensor.matmul(
                    ps[:], wt_[:, ct * D + hp * 128: ct * D + (hp + 1) * 128],
                    xap(ct, c * 512, (c + 1) * 512),
                    start=(ct == 0), stop=(ct == CT - 1))
                yield
            nc.vector.tensor_scalar_add(
                out[hp][:, c * 512:(c + 1) * 512], ps[:], bt_[:, hp:hp + 1])

        def gen_proj(c, hps=(0, 1, 2, 3), ysrc=None):
            # proj over head pairs `hps`; adds ysrc (bf16 partial) if given;
            # ships the whole chunk in one DMA when it's the final pass.
            final = hps[-1] == DT - 1
            ys = ypool.tile([128, XCH], BF16, tag="y", name="ys") if final \
                else y1pool.tile([128, XCH], BF16, tag="y1", name="ys1")
            for o in range(CT):
                ps = qkv_ps.tile([128, 512], F32, tag="qkv", name="qkvps")
                for i, hp in enumerate(hps):
                    nc.tensor.matmul(
                        ps[:], wpt[:, hp * C + o * 128: hp * C + (o + 1) * 128],
                        oT[hp][:, c * 512:(c + 1) * 512],
                        start=(i == 0), stop=(i == len(hps) - 1))
                    yield
                osl = slice(o * 512, (o + 1) * 512)
                if ysrc is None:
                    nc.vector.tensor_copy(ys[:, osl], ps[:])
                else:
                    nc.vector.tensor_add(ys[:, osl], ps[:], ysrc[:, osl])
                if final and o == CT // 2 - 1:
                    nc.scalar.dma_start(
                        yP.ap()[:, c * XCH:c * XCH + XCH // 2],
                        ys[:, 0:XCH // 2])
            if final:
                nc.scalar.dma_start(
                    yP.ap()[:, c * XCH + XCH // 2:(c + 1) * XCH],
                    ys[:, XCH // 2:])
            else:
                gen_proj.partial = ys

        # two filler queues: F (projections for next chunk; must finish
        # before that chunk's attention) and P (output proj; can linger)
        fillF = []
        fillP = []

        def pump(n):
            for _ in range(n):
                q = fillF if fillF else fillP
                if not q:
                    return
                try:
                    next(q[0])
                except StopIteration:
                    q.pop(0)

        def drain(q):
            while q:
                try:
                    next(q[0])
                except StopIteration:
                    q.pop(0)

        # ---- attention block for head pair hp, query chunk c ----
        # Diagonal j-tiles (jt = 4c+t) skip their fully-masked first 128t
        # columns in S, exp and AV; only the 128-col triangle strip is masked.
        def attention(hp, c, rate):
            njt = 4 * c + 4
            op0 = o_ps.tile([128, 512], F32, tag="o", name="ops")
            op1 = o_ps.tile([128, 512], F32, tag="o", name="ops")
            sps = {}

            def emit_S(jt):
                lo = 128 * (jt - 4 * c) if jt >= 4 * c else 0
                sp = s_ps.tile([128, 1024], F32, tag="s", name="sps")
                for half in range(2):
                    nc.tensor.matmul(
                        sp[:, half * 512 + lo:(half + 1) * 512],
                        kT[hp][half * 64:(half + 1) * 64,
                               jt * 128:(jt + 1) * 128],
                        qT[hp][half * 64:(half + 1) * 64,
                               c * 512 + lo:(c + 1) * 512],
                        start=True, stop=True)
                sps[jt] = sp

            emit_S(0)
            pcr = 0.0
            for jt in range(njt):
                if jt + 1 < njt:
                    emit_S(jt + 1)
                sp = sps.pop(jt)
                t = jt - 4 * c
                lo = 128 * t if t >= 0 else 0
                pt = ptpool.tile([128, 1024], BF16, tag="pt", name="pt")
                nc.scalar.activation(pt[:, lo:1024], sp[:, lo:1024],
                                     EXP, scale=0.125)
                if t >= 0:
                    nc.vector.tensor_mul(
                        pt[:, lo:lo + 128], pt[:, lo:lo + 128], mstrip[:])
                    nc.gpsimd.affine_select(
                        pt[:, 512 + lo:512 + lo + 128],
                        pt[:, 512 + lo:512 + lo + 128],
                        pattern=[[1, 128]],
                        compare_op=mybir.AluOpType.is_ge, fill=0.0,
                        base=0, channel_multiplier=-1)
                for half, op in ((0, op0), (1, op1)):
                    h = 2 * hp + half
                    nc.tensor.matmul(
                        op[0:65, lo:512], vt[jt][:, h * VW:h * VW + 65],
                        pt[:, half * 512 + lo:(half + 1) * 512],
                        start=(jt == 0), stop=(jt == njt - 1))
                pcr += rate
                if pcr >= 1.0:
                    k = int(pcr)
                    pump(k)
                    pcr -= k

            # normalization: stage unnormalized rows out of PSUM (releases
            # the bank), recip the rowsum straight from PSUM partition 0,
            # DMA-shift 1:65 -> oT, then scale oT in place (aligned).
            cs = slice(c * 512, (c + 1) * 512)
            for half, op, dst in ((0, op0, oT[hp][0:64, cs]),
                                  (1, op1, oT[hp][64:128, cs])):
                av = avpool.tile([128, 512], BF16, tag="avb", name="avb")
                nc.vector.tensor_copy(av[0:65, :], op[0:65, :])
                rec = recpool.tile([1, 512], F32, tag="rec", name="rec")
                nc.vector.reciprocal_approx_fast(rec[:], op[0:1, :])
                rv = rvpool.tile([128, 512], F32, tag="rv", name="rv")
                nc.gpsimd.partition_broadcast(rv[:, :], rec[0:1, :])
                nc.sync.dma_start(dst, av[1:65, :])
                nc.vector.tensor_mul(
                    dst, dst, rv[half * 64:(half + 1) * 64, :])

        # ---- main schedule: chunk-major ----
        # F_0 emitted directly (nothing to interleave with yet)
        for j in range(4):
            for _ in gen_v(j):
                pass
        for hp in range(DT):
            for which in range(2):
                for _ in gen_qk(hp, 0, which):
                    pass

        rates = [3.0, 1.2, 0.8, 0.9]
        for c in range(TC):
            if c + 1 < TC:
                for j in range(4 * (c + 1), 4 * (c + 1) + 4):
                    fillF.append(gen_v(j))
                for hp in range(DT):
                    for which in range(2):
                        fillF.append(gen_qk(hp, c + 1, which))
            for hp in range(DT):
                attention(hp, c, rates[c])
                if c == TC - 1 and hp == 2:
                    fillF.append(gen_proj(c, hps=(0, 1, 2)))
            drain(fillF)
            if c != TC - 2:
                drain(fillP)          # finish old proj before queueing new
            if c < TC - 1:
                fillP.append(gen_proj(c))
        drain(fillP)                  # proj(2) remnants cover the last norm
        for _ in gen_proj(TC - 1, hps=(3,), ysrc=gen_proj.partial):
            pass

    nc.compile()
    return nc


def _shard_inputs(x, Wk, bk, Wq, bq, Wv, bv, Wp, bp):
    bf = ml_dtypes.bfloat16

    def packw(W, sl):  # [C, D-slice] -> [128, CT*D] partition-contiguous
        return np.ascontiguousarray(
            W[:, sl].reshape(CT, 128, D).transpose(1, 0, 2)
            .reshape(128, CT * D)).astype(bf)

    in_maps = []
    for core in range(NCORES):
        b, hg = core // 2, core % 2
        sl = slice(hg * D, (hg + 1) * D)
        xb = np.asarray(x[b], np.float32)  # [T, C]
        xp = (xb.T.reshape(CT, 128, TC, 512).transpose(1, 2, 0, 3)
              .reshape(128, TC * XCH))
        wpp = (Wp[sl, :].reshape(DT, 128, C).transpose(1, 0, 2)
               .reshape(128, DT * C))
        in_maps.append({
            "xP": np.ascontiguousarray(xp).astype(bf),
            "wqP": packw(Wq, sl),
            "wkP": packw(Wk, sl),
            "wvP": packw(Wv, sl),
            "wpP": np.ascontiguousarray(wpp).astype(bf),
            "bq": np.ascontiguousarray(
                bq[sl].reshape(DT, 128).T).astype(np.float32),
            "bk": np.ascontiguousarray(
                bk[sl].reshape(DT, 128).T).astype(np.float32),
            "bv1": np.ascontiguousarray(
                bv[sl].reshape(1, D)).astype(np.float32),
        })
    return in_maps


def kernel(x, Wk, bk, Wq, bq, Wv, bv, Wp, bp, _trace=False, _trace_kwargs=None):
    x, Wk, bk, Wq, bq, Wv, bv, Wp, bp = [
        np.asarray(a) for a in (x, Wk, bk, Wq, bq, Wv, bv, Wp, bp)]
    if "nc" not in _cache:
        _cache["nc"] = _build()
    nc = _cache["nc"]
    in_maps = _shard_inputs(x, Wk, bk, Wq, bq, Wv, bv, Wp, bp)
    kw = dict(_trace_kwargs or {})
    res = run_bass_kernel_spmd(nc, in_maps, core_ids=list(range(NCORES)),
                               trace=_trace, **kw)
    out = np.empty((B, T, C), np.float32)
    for b in range(B):
        yp = (res.results[2 * b]["yP"].astype(np.float32)
              + res.results[2 * b + 1]["yP"].astype(np.float32))
        # yP[p, c*XCH + o*512 + d] = y_partial[o*128+p, c*512+d]
        yp = (yp.reshape(128, TC, CT, 512).transpose(2, 0, 1, 3)
              .reshape(C, T))
        out[b] = yp.T + bp[None, :]
    if _trace:
        _cache["last_results"] = res
    return out
